# revision 1
# baseline (speedup 1.0000x reference)
"""CRF decoder loss kernel for Trainium2 (Bass/Tile), 8-core data parallel.

Algorithm notes
---------------
The CRF forward algorithm is computed in the "hot" (exp) domain:
    u_{t+1}[j,b] = el_t[j,b] * sum_i exp(T[j,i]) * u_t[i,b]
with el_t = exp(logit_t + bias - C0).  Each step is one PE matmul
(stationary exp(T)^T, 52x52) + one DVE elementwise multiply (52x16).
A constant e^{-C0} per step keeps magnitudes near 1; every R steps a
data-dependent rescale (divide by the state-mass sum, accumulate log)
bounds fp32 range; its multiply is applied DEF steps late so it never
stalls the serial chain.  State row 51 (END) has zero incoming weights
in exp(T)^T (transition from END is -100), so it is repurposed to carry
the "end-dot" sum_i exp(T[END,i]) u_t[i] forward one step -- giving the
norm-score numerator for every prefix length without extra copies.
Per-sequence lengths select the right prefix via host-built one-hot
matrices (pure index preprocessing of `lens`/`labels`).

Emission logits are produced chunk-by-chunk with float32r matmuls and
the chunk-(ch+1) matmuls are interleaved between scan steps of chunk ch
so the in-order PE queue never head-of-line blocks the scan chain.

Gold score = sum(onehot*mask (.) logits) + sum(paircount (.) T) + label
counts (.) bias, all reduced on device.

Sharding: pure data parallel over batch (16 sequences/core); final
scalar partial losses summed on host.
"""

import numpy as np
from contextlib import ExitStack

import concourse.bass as bass
import concourse.tile as tile
from concourse import bacc
from concourse import mybir
from concourse.bass_utils import run_bass_kernel_spmd

F32 = mybir.dt.float32
F32R = mybir.dt.float32r
AF = mybir.ActivationFunctionType
ALU = mybir.AluOpType

B, S, D = 128, 512, 1024
L = 50            # real labels
NL = L + 2        # + START, END
START, END = 50, 51
NCORES = 8
BL = B // NCORES  # 16 sequences per core
TCH = 32          # timesteps per emission chunk
NCHUNK = S // TCH
KD = D // 128     # contraction chunks for emission matmul
R = 16            # rescale period (steps)
DEF = 3           # rescale apply deferral (steps)
NEV = S // R      # rescale event slots (last one unused)
C0 = 7.5          # constant per-step log damping folded into emission bias


def build_program():
    nc = bacc.Bacc("TRN2", target_bir_lowering=False, debug=False,
                   num_devices=NCORES)

    xT_d = nc.dram_tensor("xT", [KD, 128, S * BL], F32R, kind="ExternalInput")
    WT_d = nc.dram_tensor("WT", [KD, 128, L], F32R, kind="ExternalInput")
    bias_d = nc.dram_tensor("bias", [L, 1], F32, kind="ExternalInput")
    TT_d = nc.dram_tensor("TT", [NL, NL], F32, kind="ExternalInput")
    T_d = nc.dram_tensor("Tm", [NL, NL], F32, kind="ExternalInput")
    OH_d = nc.dram_tensor("OH", [L, S * BL], F32, kind="ExternalInput")
    CNT_d = nc.dram_tensor("CNT", [NL, NL], F32, kind="ExternalInput")
    CNTL_d = nc.dram_tensor("CNTL", [L, 1], F32, kind="ExternalInput")
    SELEND_d = nc.dram_tensor("SELEND", [128, 4 * BL], F32, kind="ExternalInput")
    SELSC_d = nc.dram_tensor("SELSC", [NEV + 1, BL], F32, kind="ExternalInput")
    SCC_d = nc.dram_tensor("SCC", [1, BL], F32, kind="ExternalInput")
    ELINIT_d = nc.dram_tensor("ELINIT", [2, S * BL], F32, kind="ExternalInput")
    UINIT_d = nc.dram_tensor("UINIT", [NL, BL], F32, kind="ExternalInput")
    loss_d = nc.dram_tensor("loss", [1, 1], F32, kind="ExternalOutput")
    dbg_d = nc.dram_tensor("dbg", [1, BL], F32, kind="ExternalOutput")

    with tile.TileContext(nc) as tc, ExitStack() as ctx:
        consts = ctx.enter_context(tc.tile_pool(name="consts", bufs=1))
        xpool = ctx.enter_context(tc.tile_pool(name="xpool", bufs=3))
        ohpool = ctx.enter_context(tc.tile_pool(name="ohpool", bufs=3))
        smalls = ctx.enter_context(tc.tile_pool(name="smalls", bufs=2))
        lgp = ctx.enter_context(tc.tile_pool(name="lgp", bufs=2, space="PSUM"))
        pp = ctx.enter_context(tc.tile_pool(name="pp", bufs=3, space="PSUM"))
        miscp = ctx.enter_context(tc.tile_pool(name="miscp", bufs=1, space="PSUM"))

        # ---------------- constants ----------------
        ttile = consts.tile([NL, NL], F32, name="ttile")
        nc.sync.dma_start(out=ttile[:, :], in_=TT_d.ap()[:, :])
        stat = consts.tile([NL, NL], F32, name="stat")  # stat[i,j] = exp(T[j,i])
        nc.scalar.activation(out=stat[:, :], in_=ttile[:, :], func=AF.Exp)

        wt = consts.tile([128, KD * L], F32R, name="wt")
        for k in range(KD):
            nc.sync.dma_start(out=wt[:, k * L:(k + 1) * L], in_=WT_d.ap()[k, :, :])
        braw = consts.tile([L, 1], F32, name="braw")
        nc.sync.dma_start(out=braw[:, :], in_=bias_d.ap()[:, :])
        btile = consts.tile([L, 1], F32, name="btile")
        nc.vector.tensor_scalar_add(btile[:, :], braw[:, :], -C0)

        ones = consts.tile([128, 1], F32, name="ones")
        nc.vector.memset(ones[:, :], 1.0)
        ones_r = consts.tile([1, NL], F32, name="ones_r")
        nc.vector.memset(ones_r[:, :], 1.0)

        traw = consts.tile([NL, NL], F32, name="traw")
        nc.sync.dma_start(out=traw[:, :], in_=T_d.ap()[:, :])
        cnt = consts.tile([NL, NL], F32, name="cnt")
        nc.sync.dma_start(out=cnt[:, :], in_=CNT_d.ap()[:, :])
        cntl = consts.tile([L, 1], F32, name="cntl")
        nc.sync.dma_start(out=cntl[:, :], in_=CNTL_d.ap()[:, :])
        selend = consts.tile([128, 4 * BL], F32, name="selend")
        nc.sync.dma_start(out=selend[:, :], in_=SELEND_d.ap()[:, :])
        selsc = consts.tile([NEV + 1, BL], F32, name="selsc")
        nc.sync.dma_start(out=selsc[:, :], in_=SELSC_d.ap()[:, :])

        # ---------------- big state buffers ----------------
        el_buf = consts.tile([NL, S * BL], F32, name="el_buf")
        u_buf = consts.tile([NL, (S + 2) * BL], F32, name="u_buf")
        scale_row = consts.tile([1, NEV * BL], F32, name="scale_row")
        uacc = consts.tile([L, NCHUNK], F32, name="uacc")
        scratch = consts.tile([NL, TCH * BL], F32, name="scratch")

        nc.vector.memset(scale_row[:, :], 0.0)
        nc.sync.dma_start(out=el_buf[START:START + 2, :], in_=ELINIT_d.ap()[:, :])
        nc.sync.dma_start(out=u_buf[:, 0:BL], in_=UINIT_d.ap()[:, :])

        # ---------------- emission helpers ----------------
        xt_tiles = {}
        oh_tiles = {}
        lg_tiles = {}

        def issue_dma(ch):
            xt = xpool.tile([128, KD * TCH * BL], F32R, name="xt", tag="xt")
            for k in range(KD):
                nc.sync.dma_start(out=xt[:, k * TCH * BL:(k + 1) * TCH * BL],
                                  in_=xT_d.ap()[k, :, ch * TCH * BL:(ch + 1) * TCH * BL])
            oh = ohpool.tile([L, TCH * BL], F32, name="oh", tag="oh")
            nc.sync.dma_start(out=oh[:, :],
                              in_=OH_d.ap()[:, ch * TCH * BL:(ch + 1) * TCH * BL])
            xt_tiles[ch] = xt
            oh_tiles[ch] = oh

        def em_mm(ch, k):
            if k == 0:
                lg_tiles[ch] = lgp.tile([L, TCH * BL], F32, name="lg", tag="lg")
            lg = lg_tiles[ch]
            xt = xt_tiles[ch]
            nc.tensor.matmul(
                lg[:, :],
                lhsT=wt[:, k * L:(k + 1) * L],
                rhs=xt[:, k * TCH * BL:(k + 1) * TCH * BL],
                start=(k == 0), stop=(k == KD - 1))

        def em_exp(ch):
            csl = slice(ch * TCH * BL, (ch + 1) * TCH * BL)
            nc.scalar.activation(out=el_buf[0:L, csl], in_=lg_tiles[ch][:, :],
                                 func=AF.Exp, bias=btile[:, 0:1], scale=1.0)

        def em_unary_mul(ch):
            nc.vector.tensor_mul(scratch[0:L, :], lg_tiles[ch][:, :],
                                 oh_tiles[ch][:, :])

        def em_unary_red(ch):
            nc.vector.tensor_reduce(out=uacc[:, ch:ch + 1], in_=scratch[0:L, :],
                                    axis=mybir.AxisListType.X, op=ALU.add)

        # chunk 0 emission upfront
        issue_dma(0)
        issue_dma(1)
        for k in range(KD):
            em_mm(0, k)
        em_exp(0)
        em_unary_mul(0)
        em_unary_red(0)

        # pending rescale state: (apply_step, pb_tile)
        pend_apply = {}

        # ---------------- scan with interleaved emission ----------------
        for ch in range(NCHUNK):
            if ch + 2 < NCHUNK:
                issue_dma(ch + 2)
            for tl in range(TCH):
                t = ch * TCH + tl
                p = pp.tile([NL, BL], F32, name="p", tag="p")
                nc.tensor.matmul(p[:, :], lhsT=stat[:, :],
                                 rhs=u_buf[:, t * BL:(t + 1) * BL],
                                 start=True, stop=True)
                nc.vector.tensor_mul(u_buf[:, (t + 1) * BL:(t + 2) * BL],
                                     p[:, :], el_buf[:, t * BL:(t + 1) * BL])

                # deferred rescale apply
                if t in pend_apply:
                    pb = pend_apply.pop(t)
                    nc.vector.tensor_mul(
                        u_buf[0:START, (t + 1) * BL:(t + 2) * BL],
                        u_buf[0:START, (t + 1) * BL:(t + 2) * BL],
                        pb[0:START, :])

                # rescale event: record log-sum and queue deferred apply
                if t % R == R - 1 and t + 1 + DEF <= S:
                    kev = t // R
                    ps = miscp.tile([1, BL], F32, name="ps", tag="m1")
                    nc.tensor.matmul(ps[:, :], lhsT=ones[0:L, :],
                                     rhs=u_buf[0:L, (t + 1) * BL:(t + 2) * BL],
                                     start=True, stop=True)
                    nc.scalar.activation(
                        out=scale_row[:, kev * BL:(kev + 1) * BL],
                        in_=ps[:, :], func=AF.Ln)
                    rec = smalls.tile([1, BL], F32, name="rec", tag="rec")
                    nc.vector.reciprocal(rec[:, :], ps[:, :])
                    pb = miscp.tile([NL, BL], F32, name="pb", tag="m2")
                    nc.tensor.matmul(pb[:, :], lhsT=ones_r[:, :], rhs=rec[:, :],
                                     start=True, stop=True)
                    pend_apply[t + DEF] = pb

                # interleaved emission for chunk ch+1
                if ch + 1 < NCHUNK:
                    if tl % 4 == 0:
                        em_mm(ch + 1, tl // 4)
                    elif tl == 29:
                        em_exp(ch + 1)
                    elif tl == 30:
                        em_unary_mul(ch + 1)
                    elif tl == 31:
                        em_unary_red(ch + 1)

        # final end-dot for full-length sequences (prefix L = S)
        pf = pp.tile([NL, BL], F32, name="pf", tag="p")
        nc.tensor.matmul(pf[:, :], lhsT=stat[:, :],
                         rhs=u_buf[:, S * BL:(S + 1) * BL], start=True, stop=True)
        # copy must start at a 32-aligned partition; rows 32..50 of this
        # slice are never read, only row END matters.
        nc.scalar.copy(u_buf[32:NL, (S + 1) * BL:(S + 2) * BL],
                       pf[32:NL, :])

        # ---------------- norm score selection ----------------
        endbuf = consts.tile([128, 4 * BL], F32, name="endbuf")
        for blk in range(4):
            src = u_buf[END:END + 1,
                        (blk * 128 + 2) * BL:(blk * 128 + 130) * BL]
            nc.sync.dma_start(
                out=endbuf[:, blk * BL:(blk + 1) * BL],
                in_=src.rearrange("p (q b) -> p q b", q=128, b=BL))
        nc.vector.tensor_scalar_max(endbuf[:, :], endbuf[:, :], 1e-38)
        endlog = consts.tile([128, 4 * BL], F32, name="endlog")
        nc.scalar.activation(out=endlog[:, :], in_=endbuf[:, :], func=AF.Ln)
        nc.vector.tensor_mul(endlog[:, :], endlog[:, :], selend[:, :])
        esum = consts.tile([128, BL], F32, name="esum")
        nc.vector.tensor_reduce(
            out=esum[:, :],
            in_=endlog.rearrange("p (blk b) -> p b blk", blk=4, b=BL),
            axis=mybir.AxisListType.X, op=ALU.add)

        scsel = consts.tile([NEV + 1, BL], F32, name="scsel")
        nc.sync.dma_start(out=scsel[0:NEV, :],
                          in_=scale_row.rearrange("p (k b) -> p k b", k=NEV, b=BL))
        nc.sync.dma_start(out=scsel[NEV:NEV + 1, :], in_=SCC_d.ap()[:, :])
        nc.vector.tensor_mul(scsel[:, :], scsel[:, :], selsc[:, :])

        nacc = miscp.tile([1, BL], F32, name="nacc", tag="m1")
        nc.tensor.matmul(nacc[:, :], lhsT=ones[:, :], rhs=esum[:, :],
                         start=True, stop=False)
        nc.tensor.matmul(nacc[:, :], lhsT=ones[0:NEV + 1, :], rhs=scsel[:, :],
                         start=False, stop=True)

        # ---------------- gold score ----------------
        gt1 = consts.tile([NL, 1], F32, name="gt1")
        nc.vector.tensor_mul(scratch[0:NL, 0:NL], traw[:, :], cnt[:, :])
        nc.vector.tensor_reduce(out=gt1[:, :], in_=scratch[0:NL, 0:NL],
                                axis=mybir.AxisListType.X, op=ALU.add)
        gt2 = consts.tile([L, 1], F32, name="gt2")
        nc.vector.tensor_mul(gt2[:, :], braw[:, :], cntl[:, :])
        ur = consts.tile([L, 1], F32, name="ur")
        nc.vector.tensor_reduce(out=ur[:, :], in_=uacc[:, :],
                                axis=mybir.AxisListType.X, op=ALU.add)
        gacc = miscp.tile([1, 1], F32, name="gacc", tag="m2")
        nc.tensor.matmul(gacc[:, :], lhsT=ones[0:NL, :], rhs=gt1[:, :],
                         start=True, stop=False)
        nc.tensor.matmul(gacc[:, :], lhsT=ones[0:L, :], rhs=gt2[:, :],
                         start=False, stop=False)
        nc.tensor.matmul(gacc[:, :], lhsT=ones[0:L, :], rhs=ur[:, :],
                         start=False, stop=True)

        # loss = sum_b norm - gold
        nr = smalls.tile([1, 1], F32, name="nr", tag="nr")
        nc.vector.tensor_reduce(out=nr[:, :], in_=nacc[:, :],
                                axis=mybir.AxisListType.X, op=ALU.add)
        lt = smalls.tile([1, 1], F32, name="lt", tag="lt")
        nc.vector.tensor_sub(lt[:, :], nr[:, :], gacc[:, :])
        dbgt = smalls.tile([1, BL], F32, name="dbgt", tag="dbgt")
        nc.scalar.copy(dbgt[:, :], nacc[:, :])
        nc.sync.dma_start(out=loss_d.ap()[:, :], in_=lt[:, :])
        nc.sync.dma_start(out=dbg_d.ap()[:, :], in_=dbgt[:, :])

    nc.compile()
    return nc


def prep_inputs(inputs, W, b, transition, lens, labels):
    """Host-side sharding + index preprocessing. Returns per-core input maps."""
    x = np.ascontiguousarray(np.asarray(inputs, dtype=np.float32))
    W = np.asarray(W, dtype=np.float32)
    b = np.asarray(b, dtype=np.float32)
    T = np.asarray(transition, dtype=np.float32)
    lens = np.asarray(lens).astype(np.int64)
    labels = np.asarray(labels).astype(np.int64)

    WT = np.ascontiguousarray(W.T).reshape(KD, 128, L)
    TT = np.ascontiguousarray(T.T)
    bias = b.reshape(L, 1)

    # (B,S,D) -> (D,S,B) once, then per-core contiguous slices
    xt_all = np.ascontiguousarray(np.transpose(x, (2, 1, 0)))  # (D, S, B)

    in_maps = []
    for c in range(NCORES):
        bs = slice(c * BL, (c + 1) * BL)
        lens_c = lens[bs]
        labels_c = labels[bs]

        xT = np.ascontiguousarray(xt_all[:, :, bs]).reshape(KD, 128, S * BL)

        mask = np.arange(S)[:, None] < lens_c[None, :]        # (S, BL)
        lab_t = labels_c.T                                     # (S, BL)
        OH = (lab_t[None, :, :] == np.arange(L)[:, None, None]) & mask[None]
        OH = np.ascontiguousarray(OH.astype(np.float32).reshape(L, S * BL))

        # pair counts following the reference labels_ext construction
        ext = np.full((BL, S + 2), END, dtype=np.int64)
        ext[:, 0] = START
        ext[:, 1:S + 1] = labels_c
        valid = np.arange(S + 2)[None, :] < (lens_c + 1)[:, None]
        ext = np.where(valid, ext, END)
        CNT = np.zeros((NL, NL), dtype=np.float32)
        pmask = np.arange(S + 1)[None, :] < (lens_c + 1)[:, None]
        to_ = ext[:, 1:][pmask]
        fr_ = ext[:, :-1][pmask]
        np.add.at(CNT, (to_, fr_), 1.0)

        CNTL = np.zeros((L,), dtype=np.float32)
        msk = np.arange(S)[None, :] < lens_c[:, None]
        np.add.at(CNTL, labels_c[msk], 1.0)
        CNTL = CNTL.reshape(L, 1)

        SELEND = np.zeros((128, 4 * BL), dtype=np.float32)
        q = lens_c - 1  # 0..511
        SELEND[q % 128, (q // 128) * BL + np.arange(BL)] = 1.0

        # event k (at step 16k+15) is applied to u slice 16k+16+DEF,
        # so it affects end-dots for prefix lengths >= 16k+16+DEF.
        SELSC = np.zeros((NEV + 1, BL), dtype=np.float32)
        for k in range(NEV):
            if R * k + R - 1 + 1 + DEF <= S:
                SELSC[k, :] = (lens_c >= (R * k + R + DEF)).astype(np.float32)
        SELSC[NEV, :] = 1.0
        SCC = (C0 * lens_c.astype(np.float32)).reshape(1, BL)
        ELINIT = np.zeros((2, S * BL), dtype=np.float32)
        ELINIT[1, :] = 1.0
        UINIT = np.zeros((NL, BL), dtype=np.float32)
        UINIT[START, :] = 1.0

        in_maps.append({
            "xT": xT, "WT": WT, "bias": bias, "TT": TT, "Tm": T,
            "OH": OH, "CNT": CNT, "CNTL": CNTL,
            "SELEND": SELEND, "SELSC": SELSC, "SCC": SCC,
            "ELINIT": ELINIT, "UINIT": UINIT,
        })
    return in_maps


_NC_CACHE = []


def kernel(inputs, W, b, transition, lens, labels, _trace=False, _tmpdir=None):
    in_maps = prep_inputs(inputs, W, b, transition, lens, labels)
    if not _NC_CACHE:
        _NC_CACHE.append(build_program())
    nc = _NC_CACHE[0]
    res = run_bass_kernel_spmd(nc, in_maps, list(range(NCORES)),
                               trace=_trace, tmpdir=_tmpdir)
    total = np.float64(0.0)
    for r in res.results:
        total += np.float64(r["loss"][0, 0])
    out = np.float32(total)
    if _trace:
        return out, res
    return out



# revision 2
# speedup vs baseline: 1.1118x; 1.1118x over previous
"""CRF decoder loss kernel for Trainium2 (Bass/Tile), 8-core data parallel.

Chunked warmup-probe forward scan, partition-packed
---------------------------------------------------
The CRF forward recursion u_{t+1} = el_t * (M u_t) (hot domain, M =
exp(T)^T) is a 512-step serial chain whose per-step PE->DVE round-trip
latency dominates. Products of positive matrices contract to rank-1
exponentially fast (Birkhoff), so the time axis is split into C chunks
processed CONCURRENTLY: chunk c's state starts W steps early from a
uniform probe; after W warmup steps its direction matches the true
forward state to ~1e-3 relative (loss tolerance is ~27 nats/seq, so
this is far inside budget). Magnitudes are stitched per chunk boundary
with label-mass-sum ratios (B'_{c-1} - A'_c), prefix-summed by a
lower-triangular matmul (K). Chain length: 512 -> W + G steps.

The C chunks are packed both ways: C/2 chunk-columns in the free dim
AND 2 groups in the partition dim (blockdiag(M, M) stationary,
104x104), halving the per-step DVE time. Per step: one bf16 matmul +
one DVE Hadamard. The END-row trick carries end^T M u_t for every
prefix; per-sequence lengths select norm scores via host-built
one-hots. Per-chunk rescaling (deferred apply) bounds fp16 range.

Emissions: x and W are fp8 e4m3 (W scaled by 16, duplicated in both
partition blocks so logits land in either group's rows); logits
accumulate in f32 PSUM; Act exp (scale=1/16, bias=b-C0 from host)
writes el; warmup-overlap columns are duplicated by the idle Pool
engine (Act handles the two cross-group boundaries). Gold score =
onehot unary + pair counts + bias counts, negated host-side and
accumulated into the same PSUM as the norm terms. DMAs are batched
(packed consts, 3D slice APs) because each DMA instruction occupies
the shared HWDGE descriptor generator ~625ns.
"""

import numpy as np
import ml_dtypes
from contextlib import ExitStack

import concourse.tile as tile
from concourse import bacc
from concourse import mybir
from concourse.bass_utils import run_bass_kernel_spmd

F32 = mybir.dt.float32
FP8 = mybir.dt.float8e4
BF16 = mybir.dt.bfloat16
AF = mybir.ActivationFunctionType
ALU = mybir.AluOpType

B, S, D = 128, 512, 1024
L = 50            # real labels
NL = L + 2        # + START, END
START, END = 50, 51
NCORES = 8
BL = B // NCORES  # 16 sequences per core
KD = D // 128     # contraction chunks for emission matmul

# chunked scan parameters
W = 6                 # warmup steps per probe
C = 46                # time chunks (2 partition groups x 23 free columns)
G = (S - W) // C      # valid steps per chunk (11)
assert W + C * G == S
LSTEPS = W + G        # probe chain length (17)
CH = C // 2           # chunks per partition group (23)
HB = CH * BL          # scan free width (368)
NP2 = 128             # packed partition count (two 52-row blocks)
NLB = 64              # partition base of group B's block
R = 8                 # rescale period
DEF = 3               # deferred rescale apply distance
EVENTS = [g for g in range(R - 1, LSTEPS, R) if g + DEF <= LSTEPS - 1]
NEV = len(EVENTS)     # 1 ([7])
C0 = 7.5              # per-step log damping folded into emission bias
SW = 16.0             # fp8 weight scale

NPAIRH = (LSTEPS + 1) * CH        # (slot,chunk) end-dot pairs per group (414)
BLKH = (NPAIRH + 127) // 128      # extraction blocks per group (4)
NBLK = 2 * BLKH                   # 8
TDMA = 128                        # x DMA slice (timesteps)
NSL = S // TDMA                   # 4 slices
NEM = C + 1                       # emission chunks (46 of G steps + tail W)

# packed-consts column offsets (f32 [128, CPW], one DMA)
O_TT = 0                          # T^T [NL, NL]
O_T = O_TT + NL                   # T   [NL, NL] (gold)
O_CNT = O_T + NL                  # -pair counts [NL, NL]
O_CNTL = O_CNT + NL               # -label counts [L, 1]
O_BIAS = O_CNTL + 1               # raw bias b [L, 1] (gold)
O_BIASC = O_BIAS + 1              # b - C0 at rows 0:50 AND 52:102
O_SELEND = O_BIASC + 1            # [128, NBLK*BL]
O_SELSC = O_SELEND + NBLK * BL    # [2, NEV*HB]
O_SELK = O_SELSC + NEV * HB       # [C, BL]
O_LTRI = O_SELK + BL              # [C, C]
O_SCC = O_LTRI + C                # [1, BL]
O_LT23B = O_SCC + BL              # [2, C]: row0 = +[c>=CH] (boundary fix)
O_LT23N = O_LT23B + C             # [2, C]: row1 = -[c>=CH]
O_ONES = O_LT23N + C              # [128, 1]
CPW = O_ONES + 1


def build_program():
    nc = bacc.Bacc("TRN2", target_bir_lowering=False, debug=False,
                   num_devices=NCORES)

    xT_d = nc.dram_tensor("xT", [KD, 128, S * BL], FP8, kind="ExternalInput")
    WT_d = nc.dram_tensor("WT", [KD, 128, NP2], FP8, kind="ExternalInput")
    OH_d = nc.dram_tensor("OH", [L, S * BL], BF16, kind="ExternalInput")
    UINIT_d = nc.dram_tensor("UINIT", [NP2, HB], BF16, kind="ExternalInput")
    ELI_d = nc.dram_tensor("ELI", [14, LSTEPS * HB], F32, kind="ExternalInput")
    CPK_d = nc.dram_tensor("CPK", [128, CPW], F32, kind="ExternalInput")
    BSEL_d = nc.dram_tensor("BSEL", [2, NP2], BF16, kind="ExternalInput")
    loss_d = nc.dram_tensor("loss", [1, 1 + BL], F32, kind="ExternalOutput")

    with tile.TileContext(nc) as tc, ExitStack() as ctx:
        ctx.enter_context(nc.allow_low_precision(reason="bf16 scan state"))
        consts = ctx.enter_context(tc.tile_pool(name="consts", bufs=1))
        xpool = ctx.enter_context(tc.tile_pool(name="xpool", bufs=3))
        smalls = ctx.enter_context(tc.tile_pool(name="smalls", bufs=2))
        lgp = ctx.enter_context(tc.tile_pool(name="lgp", bufs=4, space="PSUM"))
        pp = ctx.enter_context(tc.tile_pool(name="pp", bufs=2, space="PSUM"))
        miscp = ctx.enter_context(tc.tile_pool(name="miscp", bufs=1,
                                               space="PSUM"))

        # ---------------- batched input DMAs + views ----------------
        cpk = consts.tile([128, CPW], F32, name="cpk")
        nc.sync.dma_start(out=cpk[:, :], in_=CPK_d.ap()[:, :])
        ttile = cpk[0:NL, O_TT:O_TT + NL]
        traw = cpk[0:NL, O_T:O_T + NL]
        cnt = cpk[0:NL, O_CNT:O_CNT + NL]
        cntl = cpk[0:L, O_CNTL:O_CNTL + 1]
        braw = cpk[0:L, O_BIAS:O_BIAS + 1]
        selend = cpk[0:128, O_SELEND:O_SELEND + NBLK * BL]
        selsc = cpk[0:2, O_SELSC:O_SELSC + NEV * HB]
        selk = cpk[0:C, O_SELK:O_SELK + BL]
        ltri = cpk[0:C, O_LTRI:O_LTRI + C]
        scc = cpk[0:1, O_SCC:O_SCC + BL]
        lt23b = cpk[0:2, O_LT23B:O_LT23B + C]
        lt23n = cpk[0:2, O_LT23N:O_LT23N + C]
        onesr = cpk[0:128, O_ONES:O_ONES + 1]

        wt = consts.tile([128, KD * NP2], FP8, name="wt")
        nc.sync.dma_start(
            out=wt[:, :].rearrange("p (k l) -> p k l", k=KD, l=NP2),
            in_=WT_d.ap()[0:KD, :, :].rearrange("k p l -> p k l"))

        xt_tiles = {}

        def issue_dma(s):
            xt = xpool.tile([128, KD * TDMA * BL], FP8, name="xt", tag="xt")
            nc.sync.dma_start(
                out=xt[:, :].rearrange("p (k c) -> p k c", k=KD, c=TDMA * BL),
                in_=xT_d.ap()[0:KD, :, s * TDMA * BL:(s + 1) * TDMA * BL]
                .rearrange("k p c -> p k c"))
            xt_tiles[s] = xt

        issue_dma(0)   # xt slice 0 right after the small cpk/wt transfers

        oh = consts.tile([L, S * BL], BF16, name="oh")
        nc.scalar.dma_start(out=oh[:, :], in_=OH_d.ap()[:, :])

        # blockdiag stationary: statd[0:52,0:52] = statd[52:104,52:104]
        # = exp(T^T); the second block is filled by an SBUF->SBUF DMA.
        statd = consts.tile([NP2, NP2], BF16, name="statd")
        nc.vector.memset(statd[:, :], 0.0)
        nc.scalar.activation(out=statd[0:NL, 0:NL], in_=ttile, func=AF.Exp)
        nc.scalar.dma_start(out=statd[NLB:NLB + NL, NLB:NLB + NL],
                            in_=statd[0:NL, 0:NL])

        ones = consts.tile([128, 1], F32, name="ones")
        nc.vector.memset(ones[:, :], 1.0)
        # msel: matmul lhsT summing real-label rows per group -> [2, cols]
        msel = consts.tile([NP2, 2], BF16, name="msel")
        nc.vector.memset(msel[:, :], 0.0)
        nc.vector.memset(msel[0:L, 0:1], 1.0)
        nc.vector.memset(msel[NLB:NLB + L, 1:2], 1.0)
        # bsel: broadcast rec rows back to their group's label rows
        bsel = consts.tile([2, NP2], BF16, name="bsel")
        nc.scalar.dma_start(out=bsel[:, :], in_=BSEL_d.ap()[:, :])
        # e2: ones into both END rows (pb=1 there: end-dots unscaled)
        e2 = consts.tile([1, NP2], BF16, name="e2")
        nc.vector.memset(e2[:, :], 0.0)
        nc.vector.memset(e2[0:1, END:END + 1], 1.0)
        nc.vector.memset(e2[0:1, NLB + END:NLB + END + 1], 1.0)
        onesrow = consts.tile([1, HB], BF16, name="onesrow")
        nc.vector.memset(onesrow[:, :], 1.0)

        # ---------------- big state buffers ----------------
        # el_scan: per group, chunk-major [NP2, (cm*LSTEPS + g)*BL + b]
        el_scan = consts.tile([NP2, LSTEPS * HB], F32, name="el_scan")
        # u_slots: slot-major [NP2, (s*CH + cm)*BL + b]
        u_slots = consts.tile([NP2, (LSTEPS + 2) * HB], BF16, name="u_slots")
        scale_row = consts.tile([2, NEV * HB], F32, name="scale_row")
        uacc = consts.tile([L, 32], F32, name="uacc")
        scratch = consts.tile([NL, 2 * G * BL], F32, name="scratch")

        nc.vector.memset(scale_row[:, :], 0.0)
        nc.vector.memset(uacc[:, :], 0.0)
        # ELI rows: START=0, END=1, then 12 zero rows covering the
        # inter-block gap (so the Hadamard never multiplies garbage)
        nc.scalar.dma_start(out=el_scan[START:START + 14, :],
                            in_=ELI_d.ap()[:, :])
        nc.scalar.dma_start(out=el_scan[NLB + START:NLB + START + 14, :],
                            in_=ELI_d.ap()[:, :])
        nc.scalar.dma_start(out=u_slots[:, 0:HB], in_=UINIT_d.ap()[:, :])

        # gold-score constants (negated on host; accumulate into nacc)
        gscr = consts.tile([NL, NL], F32, name="gscr")
        gt1 = consts.tile([NL, 1], F32, name="gt1")
        nc.vector.tensor_mul(gscr[:, :], traw, cnt)
        nc.vector.tensor_reduce(out=gt1[:, :], in_=gscr[:, :],
                                axis=mybir.AxisListType.X, op=ALU.add)
        gt2 = consts.tile([L, 1], F32, name="gt2")
        nc.vector.tensor_mul(gt2[:, :], braw, cntl)

        issue_dma(1)

        # ---------------- emissions ----------------
        def em_chunk(c):
            t0, t1 = c  # unit = absolute time range
            lg = lgp.tile([NP2, 2 * G * BL], F32, name="lg", tag="lg")
            ranges = []
            a = t0
            while a < t1:
                b_end = min(t1, (a // TDMA + 1) * TDMA)
                ranges.append((a, b_end))
                a = b_end
            for (a, b_end) in ranges:
                s = a // TDMA
                xt = xt_tiles[s]
                co = (a - s * TDMA) * BL
                cw = (b_end - a) * BL
                for k in range(KD):
                    nc.tensor.matmul(
                        lg[:, (a - t0) * BL:(a - t0) * BL + cw],
                        lhsT=wt[:, k * NP2:(k + 1) * NP2],
                        rhs=xt[:, k * TDMA * BL + co:k * TDMA * BL + co + cw],
                        start=(k == 0), stop=(k == KD - 1))
            return lg

        def em_post(ui, unit, lg):
            t0, t1 = unit
            ncols = (t1 - t0) * BL
            c0 = t0 // G
            nck = (t1 - t0) // G     # 0 (tail), 1, or 2 scan chunks
            if nck >= 1:
                hb = c0 // CH
                cm = c0 % CH
                ro = hb * NLB
                # main exp for 1 or 2 chunks: strided 3D output AP skips
                # each chunk's warmup-tail region in el_scan
                if nck == 2:
                    dst = el_scan[ro:ro + L, cm * LSTEPS * BL:
                                  (cm + 2) * LSTEPS * BL]\
                        .rearrange("p (c x) -> p c x", c=2,
                                   x=LSTEPS * BL)[0:L, 0:2, 0:G * BL]
                    src = lg[ro:ro + L, 0:ncols].rearrange(
                        "p (c x) -> p c x", c=2, x=G * BL)
                else:
                    dst = el_scan[ro:ro + L, cm * LSTEPS * BL:
                                  cm * LSTEPS * BL + G * BL]
                    src = lg[ro:ro + L, 0:ncols]
                nc.scalar.activation(out=dst, in_=src, func=AF.Exp,
                                     bias=cpk[ro:ro + L, O_BIASC:O_BIASC + 1],
                                     scale=1.0 / SW)
                # dup for chunk c0-1 (same group: Pool; boundary: Act re-exp)
                if c0 >= 1:
                    hb2 = (c0 - 1) // CH
                    cm2 = (c0 - 1) % CH
                    ro2 = hb2 * NLB
                    ddst = el_scan[ro2:ro2 + L,
                                   (cm2 * LSTEPS + G) * BL:
                                   (cm2 * LSTEPS + G + W) * BL]
                    if hb2 == hb:
                        dsrc = el_scan[ro:ro + L, cm * LSTEPS * BL:
                                       (cm * LSTEPS) * BL + W * BL]
                        nc.gpsimd.tensor_copy(ddst, dsrc)
                    else:
                        nc.scalar.activation(
                            out=ddst, in_=lg[ro2:ro2 + L, 0:W * BL],
                            func=AF.Exp,
                            bias=cpk[ro2:ro2 + L, O_BIASC:O_BIASC + 1],
                            scale=1.0 / SW)
                if nck == 2:
                    # dup for chunk c0 from chunk c0+1's first W steps
                    ddst = el_scan[ro:ro + L,
                                   (cm * LSTEPS + G) * BL:
                                   (cm * LSTEPS + G + W) * BL]
                    dsrc = el_scan[ro:ro + L, (cm + 1) * LSTEPS * BL:
                                   (cm + 1) * LSTEPS * BL + W * BL]
                    nc.gpsimd.tensor_copy(ddst, dsrc)
            else:
                # tail [C*G, S): dup-style into last chunk of group 1
                ro = NLB
                ddst = el_scan[ro:ro + L,
                               ((CH - 1) * LSTEPS + G) * BL:
                               ((CH - 1) * LSTEPS + G + W) * BL]
                nc.scalar.activation(out=ddst, in_=lg[ro:ro + L, 0:ncols],
                                     func=AF.Exp,
                                     bias=cpk[ro:ro + L, O_BIASC:O_BIASC + 1],
                                     scale=1.0 / SW)
            # gold unary
            nc.vector.tensor_mul(scratch[0:L, 0:ncols], lg[0:L, 0:ncols],
                                 oh[:, t0 * BL:t0 * BL + ncols])
            nc.vector.tensor_reduce(out=uacc[:, ui:ui + 1],
                                    in_=scratch[0:L, 0:ncols],
                                    axis=mybir.AxisListType.X, op=ALU.add)

        # units: pairs of scan chunks within each group + singles + tail
        units = []
        for h in range(2):
            base = h * CH
            cc = base
            while cc < base + CH:
                if cc + 1 < base + CH:
                    units.append((cc * G, (cc + 2) * G))
                    cc += 2
                else:
                    units.append((cc * G, (cc + 1) * G))
                    cc += 1
        units.append((C * G, S))

        for ui, unit in enumerate(units):
            s_hi = (unit[1] - 1) // TDMA
            for s in range(len(xt_tiles), min(s_hi + 2, NSL)):
                issue_dma(s)
            lg = em_chunk(unit)
            em_post(ui, unit, lg)

        # gold unary total (PE matmuls accumulate into nacc at the end)
        ur = consts.tile([L, 1], F32, name="ur")
        nc.vector.tensor_reduce(out=ur[:, :], in_=uacc[:, :],
                                axis=mybir.AxisListType.X, op=ALU.add)
        nc.vector.tensor_scalar_mul(ur[:, :], ur[:, :], -1.0 / SW)

        # ---------------- chunked scan ----------------
        endbuf = consts.tile([128, NBLK * BL], BF16, name="endbuf")
        nc.vector.memset(endbuf[:, :], 1.0)
        a_row = consts.tile([2, HB], F32, name="a_row")

        el4 = el_scan[0:NP2, :].rearrange("p (c g b) -> p c g b",
                                          c=CH, g=LSTEPS, b=BL)

        def end_block_dma(hb, q):
            # block q of group hb: pairs (slot s>=1, cm), flat = s*CH+cm-CH
            p0 = CH + 128 * q
            p1 = min(p0 + 128, CH + NPAIRH)
            row = hb * NLB + END
            src = u_slots[row:row + 1, p0 * BL:p1 * BL]
            nc.sync.dma_start(
                out=endbuf[0:p1 - p0,
                           (hb * BLKH + q) * BL:(hb * BLKH + q + 1) * BL],
                in_=src.rearrange("p (q b) -> p q b", q=p1 - p0, b=BL))

        blk_ready = {}
        for q in range(BLKH - 1):
            blk_ready.setdefault((CH + 128 * (q + 1) - 1) // CH - 1,
                                 []).append(q)

        pend = {}
        for g in range(LSTEPS):
            p = pp.tile([NP2, HB], F32, name="p", tag="p")
            nc.tensor.matmul(p[:, :], lhsT=statd[:, :],
                             rhs=u_slots[:, g * HB:(g + 1) * HB],
                             start=True, stop=True)
            out3 = u_slots[:, (g + 1) * HB:(g + 2) * HB].rearrange(
                "p (c b) -> p c b", c=CH, b=BL)
            p3 = p[:, :].rearrange("p (c b) -> p c b", c=CH, b=BL)
            nc.vector.tensor_mul(out3, p3, el4[0:NP2, 0:CH, g, 0:BL])

            if g in pend:
                pb = pend.pop(g)
                nc.vector.tensor_mul(
                    u_slots[:, (g + 1) * HB:(g + 2) * HB],
                    u_slots[:, (g + 1) * HB:(g + 2) * HB],
                    pb[:, :])

            if g in EVENTS:
                kev = EVENTS.index(g)
                ps = miscp.tile([2, HB], F32, name="ps", tag="m1")
                nc.tensor.matmul(ps[:, :], lhsT=msel[:, :],
                                 rhs=u_slots[:, (g + 1) * HB:(g + 2) * HB],
                                 start=True, stop=True)
                nc.scalar.activation(
                    out=scale_row[:, kev * HB:(kev + 1) * HB],
                    in_=ps[:, :], func=AF.Ln)
                rec = smalls.tile([2, HB], BF16, name="rec", tag="rec")
                nc.vector.reciprocal(rec[:, :], ps[:, :])
                pb = miscp.tile([NP2, HB], F32, name="pb", tag="m2")
                nc.tensor.matmul(pb[:, :], lhsT=bsel[:, :], rhs=rec[:, :],
                                 start=True, stop=False)
                nc.tensor.matmul(pb[:, :], lhsT=e2[:, :], rhs=onesrow[:, :],
                                 start=False, stop=True)
                pend[g + DEF] = pb

            if g == W - 2:
                # boundary mass A from slot W-1 (5 warmup steps: direction
                # already converged to ~5e-3, far inside tolerance)
                bw = miscp.tile([2, HB], F32, name="bw", tag="m1")
                nc.tensor.matmul(bw[:, :], lhsT=msel[:, :],
                                 rhs=u_slots[:, (W - 1) * HB:W * HB],
                                 start=True, stop=True)
                nc.scalar.activation(out=a_row[:, :], in_=bw[:, :], func=AF.Ln)

            if g == EVENTS[0] + 1:
                # scale-log selection (scale_row complete after last event)
                scm = consts.tile([2, NEV * HB], F32, name="scm")
                nc.vector.tensor_mul(scm[:, :], scale_row[:, :], selsc)
                scred = consts.tile([2, BL], F32, name="scred")
                nc.vector.tensor_reduce(
                    out=scred[:, :],
                    in_=scm[:, :].rearrange("p (c b) -> p b c",
                                            c=NEV * CH, b=BL),
                    axis=mybir.AxisListType.X, op=ALU.add)

            if g == LSTEPS - 2:
                # stitch from slot LSTEPS-1 (same absolute boundary as the
                # A-side slot W-1), overlapping the scan tail
                be = miscp.tile([2, HB], F32, name="be", tag="m1")
                nc.tensor.matmul(be[:, :], lhsT=msel[:, :],
                                 rhs=u_slots[:, (LSTEPS - 1) * HB:LSTEPS * HB],
                                 start=True, stop=True)
                b_row = consts.tile([2, HB], F32, name="b_row")
                nc.scalar.activation(out=b_row[:, :], in_=be[:, :], func=AF.Ln)
                for kev in range(NEV):
                    nc.vector.tensor_add(b_row[:, :], b_row[:, :],
                                         scale_row[:, kev * HB:(kev + 1) * HB])
                # D in [2, HB] form; the cross-group boundary element is
                # patched into kacc by a rank-1 correction matmul (lt23)
                d2 = consts.tile([2, HB], F32, name="d2")
                nc.vector.memset(d2[:, :], 0.0)
                nc.vector.tensor_sub(d2[0:2, BL:HB], b_row[0:2, 0:HB - BL],
                                     a_row[0:2, BL:HB])
                dm = consts.tile([C, BL], F32, name="dm")
                nc.sync.dma_start(out=dm[:, :],
                                  in_=d2[:, :].rearrange(
                                      "p (c b) -> p c b", c=CH, b=BL))
                kacc = miscp.tile([C, BL], F32, name="kacc", tag="m2")
                nc.tensor.matmul(kacc[:, :], lhsT=ltri, rhs=dm[:, :],
                                 start=True, stop=False)
                nc.tensor.matmul(kacc[:, :], lhsT=lt23b,
                                 rhs=b_row[0:2, HB - BL:HB],
                                 start=False, stop=False)
                nc.tensor.matmul(kacc[:, :], lhsT=lt23n,
                                 rhs=a_row[0:2, 0:BL],
                                 start=False, stop=True)
                kmask = consts.tile([C, BL], F32, name="kmask")
                nc.vector.tensor_mul(kmask[:, :], kacc[:, :], selk)

            if g in blk_ready:
                for q in blk_ready[g]:
                    end_block_dma(0, q)
                    end_block_dma(1, q)

        # early Ln for extraction blocks that landed during the scan
        # (0..BLKH-2 of each group); the pf-dependent blocks follow later
        endlog = consts.tile([128, NBLK * BL], F32, name="endlog")
        eb = BLKH - 1
        nc.vector.tensor_scalar_max(endbuf[:, 0:eb * BL],
                                    endbuf[:, 0:eb * BL], 1e-38)
        nc.vector.tensor_scalar_max(endbuf[:, BLKH * BL:(BLKH + eb) * BL],
                                    endbuf[:, BLKH * BL:(BLKH + eb) * BL],
                                    1e-38)
        nc.scalar.activation(out=endlog[:, 0:eb * BL],
                             in_=endbuf[:, 0:eb * BL], func=AF.Ln)
        nc.scalar.activation(out=endlog[:, BLKH * BL:(BLKH + eb) * BL],
                             in_=endbuf[:, BLKH * BL:(BLKH + eb) * BL],
                             func=AF.Ln)

        # final end-dots for states at slot LSTEPS
        pf = pp.tile([NP2, HB], F32, name="pf", tag="p")
        nc.tensor.matmul(pf[:, :], lhsT=statd[:, :],
                         rhs=u_slots[:, LSTEPS * HB:(LSTEPS + 1) * HB],
                         start=True, stop=True)
        # only the END rows of the final slot matter; copy the 32-quad
        # containing each block's END row (offset starts limited to 32 rows)
        nc.scalar.copy(u_slots[32:64, (LSTEPS + 1) * HB:(LSTEPS + 2) * HB],
                       pf[32:64, :])
        nc.scalar.copy(u_slots[96:128, (LSTEPS + 1) * HB:(LSTEPS + 2) * HB],
                       pf[96:128, :])
        for hb in range(2):
            end_block_dma(hb, BLKH - 1)

        # ---------------- norm score selection ----------------
        for blk in (BLKH - 1, NBLK - 1):
            nc.vector.tensor_scalar_max(endbuf[:, blk * BL:(blk + 1) * BL],
                                        endbuf[:, blk * BL:(blk + 1) * BL],
                                        1e-38)
            nc.scalar.activation(out=endlog[:, blk * BL:(blk + 1) * BL],
                                 in_=endbuf[:, blk * BL:(blk + 1) * BL],
                                 func=AF.Ln)
        nc.vector.tensor_mul(endlog[:, :], endlog[:, :], selend)
        esum = consts.tile([128, BL], F32, name="esum")
        nc.vector.tensor_reduce(
            out=esum[:, :],
            in_=endlog[:, :].rearrange("p (blk b) -> p b blk",
                                       blk=NBLK, b=BL),
            axis=mybir.AxisListType.X, op=ALU.add)

        nacc = miscp.tile([1, BL], F32, name="nacc", tag="m1")
        nc.tensor.matmul(nacc[:, :], lhsT=onesr, rhs=esum[:, :],
                         start=True, stop=False)
        nc.tensor.matmul(nacc[:, :], lhsT=onesr[0:2, :], rhs=scred[:, :],
                         start=False, stop=False)
        nc.tensor.matmul(nacc[:, :], lhsT=onesr[0:C, :], rhs=kmask[:, :],
                         start=False, stop=False)
        nc.tensor.matmul(nacc[:, :], lhsT=onesr[0:1, :], rhs=scc,
                         start=False, stop=False)
        # negated gold pieces into column 0
        nc.tensor.matmul(nacc[:, 0:1], lhsT=ones[0:NL, :], rhs=gt1[:, :],
                         start=False, stop=False)
        nc.tensor.matmul(nacc[:, 0:1], lhsT=ones[0:L, :], rhs=gt2[:, :],
                         start=False, stop=False)
        nc.tensor.matmul(nacc[:, 0:1], lhsT=ones[0:L, :], rhs=ur[:, :],
                         start=False, stop=True)

        # loss = sum_b (norm - gold); col 0 = loss, cols 1.. = debug
        dbgt = smalls.tile([1, 1 + BL], F32, name="dbgt", tag="dbgt")
        nc.vector.tensor_reduce(out=dbgt[0:1, 0:1], in_=nacc[:, :],
                                axis=mybir.AxisListType.X, op=ALU.add)
        nc.scalar.copy(dbgt[0:1, 1:1 + BL], nacc[:, :])
        nc.sync.dma_start(out=loss_d.ap()[:, :], in_=dbgt[:, :])

    nc.compile()
    return nc


def prep_inputs(inputs, W_in, b, transition, lens, labels):
    """Host-side sharding + index preprocessing. Returns per-core input maps."""
    x = np.ascontiguousarray(np.asarray(inputs, dtype=np.float32))
    Wm = np.asarray(W_in, dtype=np.float32)
    b = np.asarray(b, dtype=np.float32)
    T = np.asarray(transition, dtype=np.float32)
    lens = np.asarray(lens).astype(np.int64)
    labels = np.asarray(labels).astype(np.int64)

    # W duplicated in both partition blocks, scaled for fp8
    WT = np.zeros((KD, 128, NP2), dtype=np.float32)
    Wk = np.ascontiguousarray((Wm * SW).T).reshape(KD, 128, L)
    WT[:, :, 0:L] = Wk
    WT[:, :, NLB:NLB + L] = Wk
    WT8 = WT.astype(ml_dtypes.float8_e4m3)
    TT = np.ascontiguousarray(T.T)

    xt_all = np.ascontiguousarray(np.transpose(x, (2, 1, 0)))  # (D, S, B)

    ELI = np.zeros((14, LSTEPS * HB), dtype=np.float32)
    ELI[1, :] = 1.0
    UINIT = np.zeros((NP2, HB), dtype=np.float32)
    UINIT[START, 0:BL] = 1.0                       # chunk 0: true init
    UINIT[:L, BL:] = 1.0 / L                       # group A probes
    UINIT[NLB:NLB + L, :] = 1.0 / L                # group B probes
    UINIT = UINIT.astype(ml_dtypes.bfloat16)
    LTRI = np.zeros((C, C), dtype=np.float32)
    for cc in range(C):
        LTRI[:cc + 1, cc] = 1.0

    in_maps = []
    for core in range(NCORES):
        bs = slice(core * BL, (core + 1) * BL)
        lens_c = lens[bs]
        labels_c = labels[bs]

        xT = np.ascontiguousarray(xt_all[:, :, bs]).reshape(KD, 128, S * BL)
        xT8 = xT.astype(ml_dtypes.float8_e4m3)

        mask = np.arange(S)[:, None] < lens_c[None, :]
        lab_t = labels_c.T
        OH = (lab_t[None, :, :] == np.arange(L)[:, None, None]) & mask[None]
        OH = np.ascontiguousarray(
            OH.reshape(L, S * BL)).astype(ml_dtypes.bfloat16)

        ext = np.full((BL, S + 2), END, dtype=np.int64)
        ext[:, 0] = START
        ext[:, 1:S + 1] = labels_c
        valid = np.arange(S + 2)[None, :] < (lens_c + 1)[:, None]
        ext = np.where(valid, ext, END)
        CNT = np.zeros((NL, NL), dtype=np.float32)
        pmask = np.arange(S + 1)[None, :] < (lens_c + 1)[:, None]
        np.add.at(CNT, (ext[:, 1:][pmask], ext[:, :-1][pmask]), 1.0)

        CNTL = np.zeros((L,), dtype=np.float32)
        msk = np.arange(S)[None, :] < lens_c[:, None]
        np.add.at(CNTL, labels_c[msk], 1.0)

        SELEND = np.zeros((128, NBLK * BL), dtype=np.float32)
        SELSC = np.zeros((2, NEV * HB), dtype=np.float32)
        SELK = np.zeros((C, BL), dtype=np.float32)
        for bb in range(BL):
            l = int(lens_c[bb])
            cch = 0 if l <= W + G else (l - W - 1) // G
            gg = l - cch * G
            hb, cm = divmod(cch, CH)
            pi = (gg + 1) * CH + cm - CH          # flat pair idx within group
            blk, row = divmod(pi, 128)
            SELEND[row, (hb * BLKH + blk) * BL + bb] = 1.0
            for kev in range(NEV):
                if EVENTS[kev] + DEF <= gg - 1:
                    SELSC[hb, (kev * CH + cm) * BL + bb] = 1.0
            SELK[cch, bb] = 1.0
        SCC = C0 * lens_c.astype(np.float32)

        CPK = np.zeros((128, CPW), dtype=np.float32)
        CPK[0:NL, O_TT:O_TT + NL] = TT
        CPK[0:NL, O_T:O_T + NL] = T
        CPK[0:NL, O_CNT:O_CNT + NL] = -CNT
        CPK[0:L, O_CNTL] = -CNTL
        CPK[0:L, O_BIAS] = b
        CPK[0:L, O_BIASC] = b - C0
        CPK[NLB:NLB + L, O_BIASC] = b - C0
        CPK[:, O_SELEND:O_SELEND + NBLK * BL] = SELEND
        CPK[0:2, O_SELSC:O_SELSC + NEV * HB] = SELSC
        CPK[0:C, O_SELK:O_SELK + BL] = SELK
        CPK[0:C, O_LTRI:O_LTRI + C] = LTRI
        CPK[0:1, O_SCC:O_SCC + BL] = SCC
        CPK[0, O_LT23B + CH:O_LT23B + C] = 1.0
        CPK[1, O_LT23N + CH:O_LT23N + C] = -1.0
        CPK[:, O_ONES] = 1.0

        BSEL = np.zeros((2, NP2), dtype=np.float32)
        BSEL[0, 0:L] = 1.0
        BSEL[1, NLB:NLB + L] = 1.0
        in_maps.append({
            "xT": xT8, "WT": WT8, "OH": OH,
            "UINIT": UINIT, "ELI": ELI, "CPK": CPK,
            "BSEL": BSEL.astype(ml_dtypes.bfloat16),
        })
    return in_maps


_NC_CACHE = []


def kernel(inputs, W, b, transition, lens, labels, _trace=False, _tmpdir=None):
    in_maps = prep_inputs(inputs, W, b, transition, lens, labels)
    if not _NC_CACHE:
        _NC_CACHE.append(build_program())
    nc = _NC_CACHE[0]
    res = run_bass_kernel_spmd(nc, in_maps, list(range(NCORES)),
                               trace=_trace, tmpdir=_tmpdir)
    total = np.float64(0.0)
    for r in res.results:
        total += np.float64(r["loss"][0, 0])
    out = np.float32(total)
    if _trace:
        return out, res
    return out


# revision 4
# speedup vs baseline: 1.1368x; 1.0225x over previous
"""CRF decoder loss kernel for Trainium2 (Bass/Tile), 8-core data parallel.

Chunked warmup-probe forward scan, partition-packed
---------------------------------------------------
The CRF forward recursion u_{t+1} = el_t * (M u_t) (hot domain, M =
exp(T)^T) is a 512-step serial chain whose per-step PE->DVE round-trip
latency dominates. Products of positive matrices contract to rank-1
exponentially fast (Birkhoff), so the time axis is split into C chunks
processed CONCURRENTLY: chunk c's state starts W steps early from a
uniform probe; after W warmup steps its direction matches the true
forward state to ~1e-3 relative (loss tolerance is ~27 nats/seq, so
this is far inside budget). Magnitudes are stitched per chunk boundary
with label-mass-sum ratios (B'_{c-1} - A'_c), prefix-summed by a
lower-triangular matmul (K). Chain length: 512 -> W + G steps.

The C chunks are packed both ways: C/2 chunk-columns in the free dim
AND 2 groups in the partition dim (blockdiag(M, M) stationary,
104x104), halving the per-step DVE time. Per step: one bf16 matmul +
one DVE Hadamard. The END-row trick carries end^T M u_t for every
prefix; per-sequence lengths select norm scores via host-built
one-hots. Per-chunk rescaling (deferred apply) bounds fp16 range.

Emissions: x and W are fp8 e4m3 (W scaled by 16, duplicated in both
partition blocks so logits land in either group's rows); logits
accumulate in f32 PSUM; Act exp (scale=1/16, bias=b-C0 from host)
writes el; warmup-overlap columns are duplicated by the idle Pool
engine (Act handles the two cross-group boundaries). Gold score =
onehot unary + pair counts + bias counts, negated host-side and
accumulated into the same PSUM as the norm terms. DMAs are batched
(packed consts, 3D slice APs) because each DMA instruction occupies
the shared HWDGE descriptor generator ~625ns.
"""

import numpy as np
import ml_dtypes
from contextlib import ExitStack

import concourse.tile as tile
from concourse import bacc
from concourse import mybir
from concourse.bass_utils import run_bass_kernel_spmd

F32 = mybir.dt.float32
FP8 = mybir.dt.float8e4
BF16 = mybir.dt.bfloat16
AF = mybir.ActivationFunctionType
ALU = mybir.AluOpType

B, S, D = 128, 512, 1024
L = 50            # real labels
NL = L + 2        # + START, END
START, END = 50, 51
NCORES = 8
BL = B // NCORES  # 16 sequences per core
KD = D // 128     # contraction chunks for emission matmul

# chunked scan parameters
W = 6                 # warmup steps per probe
C = 46                # time chunks (2 partition groups x 23 free columns)
G = (S - W) // C      # valid steps per chunk (11)
assert W + C * G == S
LSTEPS = W + G        # probe chain length (17)
CH = C // 2           # chunks per partition group (23)
HB = CH * BL          # scan free width (368)
NP2 = 128             # packed partition count (two 52-row blocks)
NLB = 64              # partition base of group B's block
R = 8                 # rescale period
DEF = 3               # deferred rescale apply distance
EVENTS = [g for g in range(R - 1, LSTEPS, R) if g + DEF <= LSTEPS - 1]
NEV = len(EVENTS)     # 1 ([7])
C0 = 7.5              # per-step log damping folded into emission bias
SW = 16.0             # fp8 weight scale

NPAIRH = (LSTEPS + 1) * CH        # (slot,chunk) end-dot pairs per group (414)
BLKH = (NPAIRH + 127) // 128      # extraction blocks per group (4)
NBLK = 2 * BLKH                   # 8
TDMA = 128                        # x DMA slice (timesteps)
NSL = S // TDMA                   # 4 slices
NEM = C + 1                       # emission chunks (46 of G steps + tail W)

# packed-consts column offsets (f32 [128, CPW], one DMA)
O_TT = 0                          # T^T [NL, NL]
O_T = O_TT + NL                   # T   [NL, NL] (gold)
O_CNT = O_T + NL                  # -pair counts [NL, NL]
O_CNTL = O_CNT + NL               # -label counts [L, 1]
O_BIAS = O_CNTL + 1               # raw bias b [L, 1] (gold)
O_BIASC = O_BIAS + 1              # b - C0 at rows 0:50 AND 52:102
O_SELEND = O_BIASC + 1            # [128, NBLK*BL]
O_SELSC = O_SELEND + NBLK * BL    # [2, NEV*HB]
O_SELK = O_SELSC + NEV * HB       # [C, BL]
O_LTRI = O_SELK + BL              # [C, C]
O_SCC = O_LTRI + C                # [1, BL]
O_LT23B = O_SCC + BL              # [2, C]: row0 = +[c>=CH] (boundary fix)
O_LT23N = O_LT23B + C             # [2, C]: row1 = -[c>=CH]
O_ONES = O_LT23N + C              # [128, 1]
CPW = O_ONES + 1


def build_program():
    nc = bacc.Bacc("TRN2", target_bir_lowering=False, debug=False,
                   num_devices=NCORES)

    xT_d = nc.dram_tensor("xT", [KD, 128, S * BL], FP8, kind="ExternalInput")
    WT_d = nc.dram_tensor("WT", [KD, 128, NP2], FP8, kind="ExternalInput")
    OH_d = nc.dram_tensor("OH", [L, S * BL], BF16, kind="ExternalInput")
    UINIT_d = nc.dram_tensor("UINIT", [NP2, HB], BF16, kind="ExternalInput")
    ELI_d = nc.dram_tensor("ELI", [14, LSTEPS * HB], F32, kind="ExternalInput")
    CPK_d = nc.dram_tensor("CPK", [128, CPW], F32, kind="ExternalInput")
    BSEL_d = nc.dram_tensor("BSEL", [2, NP2], BF16, kind="ExternalInput")
    loss_d = nc.dram_tensor("loss", [1, 1 + BL], F32, kind="ExternalOutput")

    with tile.TileContext(nc) as tc, ExitStack() as ctx:
        ctx.enter_context(nc.allow_low_precision(reason="bf16 scan state"))
        consts = ctx.enter_context(tc.tile_pool(name="consts", bufs=1))
        xpool = ctx.enter_context(tc.tile_pool(name="xpool", bufs=1))
        smalls = ctx.enter_context(tc.tile_pool(name="smalls", bufs=2))
        lgp = ctx.enter_context(tc.tile_pool(name="lgp", bufs=4, space="PSUM"))
        pp = ctx.enter_context(tc.tile_pool(name="pp", bufs=2, space="PSUM"))
        miscp = ctx.enter_context(tc.tile_pool(name="miscp", bufs=1,
                                               space="PSUM"))

        # ---------------- batched input DMAs + views ----------------
        cpk = consts.tile([128, CPW], F32, name="cpk")
        nc.sync.dma_start(out=cpk[:, :], in_=CPK_d.ap()[:, :])
        ttile = cpk[0:NL, O_TT:O_TT + NL]
        traw = cpk[0:NL, O_T:O_T + NL]
        cnt = cpk[0:NL, O_CNT:O_CNT + NL]
        cntl = cpk[0:L, O_CNTL:O_CNTL + 1]
        braw = cpk[0:L, O_BIAS:O_BIAS + 1]
        selend = cpk[0:128, O_SELEND:O_SELEND + NBLK * BL]
        selsc = cpk[0:2, O_SELSC:O_SELSC + NEV * HB]
        selk = cpk[0:C, O_SELK:O_SELK + BL]
        ltri = cpk[0:C, O_LTRI:O_LTRI + C]
        scc = cpk[0:1, O_SCC:O_SCC + BL]
        lt23b = cpk[0:2, O_LT23B:O_LT23B + C]
        lt23n = cpk[0:2, O_LT23N:O_LT23N + C]
        onesr = cpk[0:128, O_ONES:O_ONES + 1]

        wt = consts.tile([128, KD * NP2], FP8, name="wt")
        nc.sync.dma_start(
            out=wt[:, :].rearrange("p (k l) -> p k l", k=KD, l=NP2),
            in_=WT_d.ap()[0:KD, :, :].rearrange("k p l -> p k l"))

        # x slices with finer granularity at the start so the first
        # emission units start as soon as possible
        SLICES = [(0, 32), (32, 64), (64, 128), (128, 256), (256, 384),
                  (384, 512)]

        def slice_of(t):
            for i, (a, b) in enumerate(SLICES):
                if a <= t < b:
                    return i
            raise AssertionError(t)

        xt_tiles = {}

        def issue_dma(si):
            s0, s1 = SLICES[si]
            cols = (s1 - s0) * BL
            xt = xpool.tile([128, KD * cols], FP8, name="xt", tag=f"xt{si}")
            nc.sync.dma_start(
                out=xt[:, :].rearrange("p (k c) -> p k c", k=KD, c=cols),
                in_=xT_d.ap()[0:KD, :, s0 * BL:s1 * BL]
                .rearrange("k p c -> p k c"))
            xt_tiles[si] = xt

        issue_dma(0)

        oh = consts.tile([L, S * BL], BF16, name="oh")
        nc.scalar.dma_start(out=oh[:, :], in_=OH_d.ap()[:, :])

        # blockdiag stationary: statd[0:52,0:52] = statd[52:104,52:104]
        # = exp(T^T); the second block is filled by an SBUF->SBUF DMA.
        statd = consts.tile([NP2, NP2], BF16, name="statd")
        nc.vector.memset(statd[:, :], 0.0)
        nc.scalar.activation(out=statd[0:NL, 0:NL], in_=ttile, func=AF.Exp)
        nc.scalar.dma_start(out=statd[NLB:NLB + NL, NLB:NLB + NL],
                            in_=statd[0:NL, 0:NL])

        ones = consts.tile([128, 1], F32, name="ones")
        nc.vector.memset(ones[:, :], 1.0)
        # msel: matmul lhsT summing real-label rows per group -> [2, cols]
        msel = consts.tile([NP2, 2], BF16, name="msel")
        nc.vector.memset(msel[:, :], 0.0)
        nc.vector.memset(msel[0:L, 0:1], 1.0)
        nc.vector.memset(msel[NLB:NLB + L, 1:2], 1.0)
        # bsel: broadcast rec rows back to their group's label rows
        bsel = consts.tile([2, NP2], BF16, name="bsel")
        nc.scalar.dma_start(out=bsel[:, :], in_=BSEL_d.ap()[:, :])
        # e2: ones into both END rows (pb=1 there: end-dots unscaled)
        e2 = consts.tile([1, NP2], BF16, name="e2")
        nc.vector.memset(e2[:, :], 0.0)
        nc.vector.memset(e2[0:1, END:END + 1], 1.0)
        nc.vector.memset(e2[0:1, NLB + END:NLB + END + 1], 1.0)
        onesrow = consts.tile([1, HB], BF16, name="onesrow")
        nc.vector.memset(onesrow[:, :], 1.0)

        # ---------------- big state buffers ----------------
        # el_scan: per group, chunk-major [NP2, (cm*LSTEPS + g)*BL + b]
        el_scan = consts.tile([NP2, LSTEPS * HB], F32, name="el_scan")
        # u_slots: slot-major [NP2, (s*CH + cm)*BL + b]
        u_slots = consts.tile([NP2, (LSTEPS + 2) * HB], BF16, name="u_slots")
        scale_row = consts.tile([2, NEV * HB], F32, name="scale_row")
        uacc = consts.tile([L, 32], F32, name="uacc")
        scratch = consts.tile([NL, 2 * G * BL], F32, name="scratch")

        nc.vector.memset(scale_row[:, :], 0.0)
        nc.vector.memset(uacc[:, :], 0.0)
        # ELI rows: START=0, END=1, then 12 zero rows covering the
        # inter-block gap (so the Hadamard never multiplies garbage)
        nc.scalar.dma_start(out=el_scan[START:START + 14, :],
                            in_=ELI_d.ap()[:, :])
        nc.scalar.dma_start(out=el_scan[NLB + START:NLB + START + 14, :],
                            in_=ELI_d.ap()[:, :])
        nc.scalar.dma_start(out=u_slots[:, 0:HB], in_=UINIT_d.ap()[:, :])

        # gold-score constants (negated on host; accumulate into nacc)
        gscr = consts.tile([NL, NL], F32, name="gscr")
        gt1 = consts.tile([NL, 1], F32, name="gt1")
        nc.vector.tensor_mul(gscr[:, :], traw, cnt)
        nc.vector.tensor_reduce(out=gt1[:, :], in_=gscr[:, :],
                                axis=mybir.AxisListType.X, op=ALU.add)
        gt2 = consts.tile([L, 1], F32, name="gt2")
        nc.vector.tensor_mul(gt2[:, :], braw, cntl)

        issue_dma(1)

        # ---------------- emissions ----------------
        def em_chunk(c):
            t0, t1 = c  # unit = absolute time range
            lg = lgp.tile([NP2, 2 * G * BL], F32, name="lg", tag="lg")
            ranges = []
            a = t0
            while a < t1:
                b_end = min(t1, SLICES[slice_of(a)][1])
                ranges.append((a, b_end))
                a = b_end
            for (a, b_end) in ranges:
                si = slice_of(a)
                s0, s1 = SLICES[si]
                xt = xt_tiles[si]
                sb = (s1 - s0) * BL
                co = (a - s0) * BL
                cw = (b_end - a) * BL
                for k in range(KD):
                    nc.tensor.matmul(
                        lg[:, (a - t0) * BL:(a - t0) * BL + cw],
                        lhsT=wt[:, k * NP2:(k + 1) * NP2],
                        rhs=xt[:, k * sb + co:k * sb + co + cw],
                        start=(k == 0), stop=(k == KD - 1))
            return lg

        def em_post(ui, unit, lg):
            t0, t1 = unit
            ncols = (t1 - t0) * BL
            c0 = t0 // G
            nck = (t1 - t0) // G     # 0 (tail), 1, or 2 scan chunks
            if nck >= 1:
                hb = c0 // CH
                cm = c0 % CH
                ro = hb * NLB
                # main exp for 1 or 2 chunks: strided 3D output AP skips
                # each chunk's warmup-tail region in el_scan
                if nck == 2:
                    dst = el_scan[ro:ro + L, cm * LSTEPS * BL:
                                  (cm + 2) * LSTEPS * BL]\
                        .rearrange("p (c x) -> p c x", c=2,
                                   x=LSTEPS * BL)[0:L, 0:2, 0:G * BL]
                    src = lg[ro:ro + L, 0:ncols].rearrange(
                        "p (c x) -> p c x", c=2, x=G * BL)
                else:
                    dst = el_scan[ro:ro + L, cm * LSTEPS * BL:
                                  cm * LSTEPS * BL + G * BL]
                    src = lg[ro:ro + L, 0:ncols]
                nc.scalar.activation(out=dst, in_=src, func=AF.Exp,
                                     bias=cpk[ro:ro + L, O_BIASC:O_BIASC + 1],
                                     scale=1.0 / SW)
                # dup for chunk c0-1 (same group: Pool; boundary: Act re-exp)
                if c0 >= 1:
                    hb2 = (c0 - 1) // CH
                    cm2 = (c0 - 1) % CH
                    ro2 = hb2 * NLB
                    ddst = el_scan[ro2:ro2 + L,
                                   (cm2 * LSTEPS + G) * BL:
                                   (cm2 * LSTEPS + G + W) * BL]
                    if hb2 == hb:
                        dsrc = el_scan[ro:ro + L, cm * LSTEPS * BL:
                                       (cm * LSTEPS) * BL + W * BL]
                        nc.gpsimd.tensor_copy(ddst, dsrc)
                    else:
                        nc.scalar.activation(
                            out=ddst, in_=lg[ro2:ro2 + L, 0:W * BL],
                            func=AF.Exp,
                            bias=cpk[ro2:ro2 + L, O_BIASC:O_BIASC + 1],
                            scale=1.0 / SW)
                if nck == 2:
                    # dup for chunk c0 from chunk c0+1's first W steps
                    ddst = el_scan[ro:ro + L,
                                   (cm * LSTEPS + G) * BL:
                                   (cm * LSTEPS + G + W) * BL]
                    dsrc = el_scan[ro:ro + L, (cm + 1) * LSTEPS * BL:
                                   (cm + 1) * LSTEPS * BL + W * BL]
                    nc.gpsimd.tensor_copy(ddst, dsrc)
            else:
                # tail [C*G, S): dup-style into last chunk of group 1
                ro = NLB
                ddst = el_scan[ro:ro + L,
                               ((CH - 1) * LSTEPS + G) * BL:
                               ((CH - 1) * LSTEPS + G + W) * BL]
                nc.scalar.activation(out=ddst, in_=lg[ro:ro + L, 0:ncols],
                                     func=AF.Exp,
                                     bias=cpk[ro:ro + L, O_BIASC:O_BIASC + 1],
                                     scale=1.0 / SW)
            # gold unary
            nc.vector.tensor_mul(scratch[0:L, 0:ncols], lg[0:L, 0:ncols],
                                 oh[:, t0 * BL:t0 * BL + ncols])
            nc.vector.tensor_reduce(out=uacc[:, ui:ui + 1],
                                    in_=scratch[0:L, 0:ncols],
                                    axis=mybir.AxisListType.X, op=ALU.add)

        # units: pairs of scan chunks within each group + singles + tail
        units = []
        for h in range(2):
            base = h * CH
            cc = base
            while cc < base + CH:
                if cc + 1 < base + CH:
                    units.append((cc * G, (cc + 2) * G))
                    cc += 2
                else:
                    units.append((cc * G, (cc + 1) * G))
                    cc += 1
        units.append((C * G, S))

        for ui, unit in enumerate(units):
            s_hi = slice_of(unit[1] - 1)
            for si in range(len(xt_tiles), min(s_hi + 2, len(SLICES))):
                issue_dma(si)
            lg = em_chunk(unit)
            em_post(ui, unit, lg)

        # gold unary total (PE matmuls accumulate into nacc at the end)
        ur = consts.tile([L, 1], F32, name="ur")
        nc.vector.tensor_reduce(out=ur[:, :], in_=uacc[:, :],
                                axis=mybir.AxisListType.X, op=ALU.add)
        nc.vector.tensor_scalar_mul(ur[:, :], ur[:, :], -1.0 / SW)

        # ---------------- chunked scan ----------------
        endbuf = consts.tile([128, NBLK * BL], BF16, name="endbuf")
        nc.vector.memset(endbuf[:, :], 1.0)
        a_row = consts.tile([2, HB], F32, name="a_row")

        el4 = el_scan[0:NP2, :].rearrange("p (c g b) -> p c g b",
                                          c=CH, g=LSTEPS, b=BL)

        def end_block_dma(hb, q):
            # block q of group hb: pairs (slot s>=1, cm), flat = s*CH+cm-CH
            p0 = CH + 128 * q
            p1 = min(p0 + 128, CH + NPAIRH)
            row = hb * NLB + END
            src = u_slots[row:row + 1, p0 * BL:p1 * BL]
            nc.sync.dma_start(
                out=endbuf[0:p1 - p0,
                           (hb * BLKH + q) * BL:(hb * BLKH + q + 1) * BL],
                in_=src.rearrange("p (q b) -> p q b", q=p1 - p0, b=BL))

        blk_ready = {}
        for q in range(BLKH - 1):
            blk_ready.setdefault((CH + 128 * (q + 1) - 1) // CH - 1,
                                 []).append(q)

        pend = {}
        for g in range(LSTEPS):
            p = pp.tile([NP2, HB], F32, name="p", tag="p")
            nc.tensor.matmul(p[:, :], lhsT=statd[:, :],
                             rhs=u_slots[:, g * HB:(g + 1) * HB],
                             start=True, stop=True)
            out3 = u_slots[:, (g + 1) * HB:(g + 2) * HB].rearrange(
                "p (c b) -> p c b", c=CH, b=BL)
            p3 = p[:, :].rearrange("p (c b) -> p c b", c=CH, b=BL)
            nc.vector.tensor_mul(out3, p3, el4[0:NP2, 0:CH, g, 0:BL])

            if g in pend:
                pb = pend.pop(g)
                nc.vector.tensor_mul(
                    u_slots[:, (g + 1) * HB:(g + 2) * HB],
                    u_slots[:, (g + 1) * HB:(g + 2) * HB],
                    pb[:, :])

            if g in EVENTS:
                kev = EVENTS.index(g)
                ps = miscp.tile([2, HB], F32, name="ps", tag="m1")
                nc.tensor.matmul(ps[:, :], lhsT=msel[:, :],
                                 rhs=u_slots[:, (g + 1) * HB:(g + 2) * HB],
                                 start=True, stop=True)
                nc.scalar.activation(
                    out=scale_row[:, kev * HB:(kev + 1) * HB],
                    in_=ps[:, :], func=AF.Ln)
                rec = smalls.tile([2, HB], BF16, name="rec", tag="rec")
                nc.vector.reciprocal(rec[:, :], ps[:, :])
                pb = miscp.tile([NP2, HB], F32, name="pb", tag="m2")
                nc.tensor.matmul(pb[:, :], lhsT=bsel[:, :], rhs=rec[:, :],
                                 start=True, stop=False)
                nc.tensor.matmul(pb[:, :], lhsT=e2[:, :], rhs=onesrow[:, :],
                                 start=False, stop=True)
                pend[g + DEF] = pb

            if g == W - 2:
                # boundary mass A from slot W-1 (5 warmup steps: direction
                # already converged to ~5e-3, far inside tolerance)
                bw = miscp.tile([2, HB], F32, name="bw", tag="m1")
                nc.tensor.matmul(bw[:, :], lhsT=msel[:, :],
                                 rhs=u_slots[:, (W - 1) * HB:W * HB],
                                 start=True, stop=True)
                nc.scalar.activation(out=a_row[:, :], in_=bw[:, :], func=AF.Ln)

            if g == EVENTS[0] + 1:
                # scale-log selection (scale_row complete after last event)
                scm = consts.tile([2, NEV * HB], F32, name="scm")
                nc.vector.tensor_mul(scm[:, :], scale_row[:, :], selsc)
                scred = consts.tile([2, BL], F32, name="scred")
                nc.vector.tensor_reduce(
                    out=scred[:, :],
                    in_=scm[:, :].rearrange("p (c b) -> p b c",
                                            c=NEV * CH, b=BL),
                    axis=mybir.AxisListType.X, op=ALU.add)

            if g == LSTEPS - 2:
                # stitch from slot LSTEPS-1 (same absolute boundary as the
                # A-side slot W-1), overlapping the scan tail
                be = miscp.tile([2, HB], F32, name="be", tag="m1")
                nc.tensor.matmul(be[:, :], lhsT=msel[:, :],
                                 rhs=u_slots[:, (LSTEPS - 1) * HB:LSTEPS * HB],
                                 start=True, stop=True)
                b_row = consts.tile([2, HB], F32, name="b_row")
                nc.scalar.activation(out=b_row[:, :], in_=be[:, :], func=AF.Ln)
                for kev in range(NEV):
                    nc.vector.tensor_add(b_row[:, :], b_row[:, :],
                                         scale_row[:, kev * HB:(kev + 1) * HB])
                # D in [2, HB] form; the cross-group boundary element is
                # patched into kacc by a rank-1 correction matmul (lt23)
                d2 = consts.tile([2, HB], F32, name="d2")
                nc.vector.memset(d2[:, :], 0.0)
                nc.vector.tensor_sub(d2[0:2, BL:HB], b_row[0:2, 0:HB - BL],
                                     a_row[0:2, BL:HB])
                dm = consts.tile([C, BL], F32, name="dm")
                nc.sync.dma_start(out=dm[:, :],
                                  in_=d2[:, :].rearrange(
                                      "p (c b) -> p c b", c=CH, b=BL))
                kacc = miscp.tile([C, BL], F32, name="kacc", tag="m2")
                nc.tensor.matmul(kacc[:, :], lhsT=ltri, rhs=dm[:, :],
                                 start=True, stop=False)
                nc.tensor.matmul(kacc[:, :], lhsT=lt23b,
                                 rhs=b_row[0:2, HB - BL:HB],
                                 start=False, stop=False)
                nc.tensor.matmul(kacc[:, :], lhsT=lt23n,
                                 rhs=a_row[0:2, 0:BL],
                                 start=False, stop=True)
                kmask = consts.tile([C, BL], F32, name="kmask")
                nc.vector.tensor_mul(kmask[:, :], kacc[:, :], selk)

            if g in blk_ready:
                for q in blk_ready[g]:
                    end_block_dma(0, q)
                    end_block_dma(1, q)

        # early Ln for extraction blocks that landed during the scan
        # (0..BLKH-2 of each group); the pf-dependent blocks follow later
        endlog = consts.tile([128, NBLK * BL], F32, name="endlog")
        eb = BLKH - 1
        nc.vector.tensor_scalar_max(endbuf[:, 0:eb * BL],
                                    endbuf[:, 0:eb * BL], 1e-38)
        nc.vector.tensor_scalar_max(endbuf[:, BLKH * BL:(BLKH + eb) * BL],
                                    endbuf[:, BLKH * BL:(BLKH + eb) * BL],
                                    1e-38)
        nc.scalar.activation(out=endlog[:, 0:eb * BL],
                             in_=endbuf[:, 0:eb * BL], func=AF.Ln)
        nc.scalar.activation(out=endlog[:, BLKH * BL:(BLKH + eb) * BL],
                             in_=endbuf[:, BLKH * BL:(BLKH + eb) * BL],
                             func=AF.Ln)

        # final end-dots for states at slot LSTEPS
        pf = pp.tile([NP2, HB], F32, name="pf", tag="p")
        nc.tensor.matmul(pf[:, :], lhsT=statd[:, :],
                         rhs=u_slots[:, LSTEPS * HB:(LSTEPS + 1) * HB],
                         start=True, stop=True)
        # only the END rows of the final slot matter; copy the 32-quad
        # containing each block's END row (offset starts limited to 32 rows)
        nc.scalar.copy(u_slots[32:64, (LSTEPS + 1) * HB:(LSTEPS + 2) * HB],
                       pf[32:64, :])
        nc.scalar.copy(u_slots[96:128, (LSTEPS + 1) * HB:(LSTEPS + 2) * HB],
                       pf[96:128, :])
        for hb in range(2):
            end_block_dma(hb, BLKH - 1)

        # ---------------- norm score selection ----------------
        for blk in (BLKH - 1, NBLK - 1):
            nc.vector.tensor_scalar_max(endbuf[:, blk * BL:(blk + 1) * BL],
                                        endbuf[:, blk * BL:(blk + 1) * BL],
                                        1e-38)
            nc.scalar.activation(out=endlog[:, blk * BL:(blk + 1) * BL],
                                 in_=endbuf[:, blk * BL:(blk + 1) * BL],
                                 func=AF.Ln)
        nc.vector.tensor_mul(endlog[:, :], endlog[:, :], selend)
        esum = consts.tile([128, BL], F32, name="esum")
        nc.vector.tensor_reduce(
            out=esum[:, :],
            in_=endlog[:, :].rearrange("p (blk b) -> p b blk",
                                       blk=NBLK, b=BL),
            axis=mybir.AxisListType.X, op=ALU.add)

        nacc = miscp.tile([1, BL], F32, name="nacc", tag="m1")
        nc.tensor.matmul(nacc[:, :], lhsT=onesr, rhs=esum[:, :],
                         start=True, stop=False)
        nc.tensor.matmul(nacc[:, :], lhsT=onesr[0:2, :], rhs=scred[:, :],
                         start=False, stop=False)
        nc.tensor.matmul(nacc[:, :], lhsT=onesr[0:C, :], rhs=kmask[:, :],
                         start=False, stop=False)
        nc.tensor.matmul(nacc[:, :], lhsT=onesr[0:1, :], rhs=scc,
                         start=False, stop=False)
        # negated gold pieces into column 0
        nc.tensor.matmul(nacc[:, 0:1], lhsT=ones[0:NL, :], rhs=gt1[:, :],
                         start=False, stop=False)
        nc.tensor.matmul(nacc[:, 0:1], lhsT=ones[0:L, :], rhs=gt2[:, :],
                         start=False, stop=False)
        nc.tensor.matmul(nacc[:, 0:1], lhsT=ones[0:L, :], rhs=ur[:, :],
                         start=False, stop=True)

        # loss = sum_b (norm - gold); col 0 = loss, cols 1.. = debug
        dbgt = smalls.tile([1, 1 + BL], F32, name="dbgt", tag="dbgt")
        nc.vector.tensor_reduce(out=dbgt[0:1, 0:1], in_=nacc[:, :],
                                axis=mybir.AxisListType.X, op=ALU.add)
        nc.scalar.copy(dbgt[0:1, 1:1 + BL], nacc[:, :])
        nc.sync.dma_start(out=loss_d.ap()[:, :], in_=dbgt[:, :])

    nc.compile()
    return nc


def prep_inputs(inputs, W_in, b, transition, lens, labels):
    """Host-side sharding + index preprocessing. Returns per-core input maps."""
    x = np.ascontiguousarray(np.asarray(inputs, dtype=np.float32))
    Wm = np.asarray(W_in, dtype=np.float32)
    b = np.asarray(b, dtype=np.float32)
    T = np.asarray(transition, dtype=np.float32)
    lens = np.asarray(lens).astype(np.int64)
    labels = np.asarray(labels).astype(np.int64)

    # W duplicated in both partition blocks, scaled for fp8
    WT = np.zeros((KD, 128, NP2), dtype=np.float32)
    Wk = np.ascontiguousarray((Wm * SW).T).reshape(KD, 128, L)
    WT[:, :, 0:L] = Wk
    WT[:, :, NLB:NLB + L] = Wk
    WT8 = WT.astype(ml_dtypes.float8_e4m3)
    TT = np.ascontiguousarray(T.T)

    xt_all = np.ascontiguousarray(np.transpose(x, (2, 1, 0)))  # (D, S, B)

    ELI = np.zeros((14, LSTEPS * HB), dtype=np.float32)
    ELI[1, :] = 1.0
    UINIT = np.zeros((NP2, HB), dtype=np.float32)
    UINIT[START, 0:BL] = 1.0                       # chunk 0: true init
    UINIT[:L, BL:] = 1.0 / L                       # group A probes
    UINIT[NLB:NLB + L, :] = 1.0 / L                # group B probes
    UINIT = UINIT.astype(ml_dtypes.bfloat16)
    LTRI = np.zeros((C, C), dtype=np.float32)
    for cc in range(C):
        LTRI[:cc + 1, cc] = 1.0

    in_maps = []
    for core in range(NCORES):
        bs = slice(core * BL, (core + 1) * BL)
        lens_c = lens[bs]
        labels_c = labels[bs]

        xT = np.ascontiguousarray(xt_all[:, :, bs]).reshape(KD, 128, S * BL)
        xT8 = xT.astype(ml_dtypes.float8_e4m3)

        mask = np.arange(S)[:, None] < lens_c[None, :]
        lab_t = labels_c.T
        OH = (lab_t[None, :, :] == np.arange(L)[:, None, None]) & mask[None]
        OH = np.ascontiguousarray(
            OH.reshape(L, S * BL)).astype(ml_dtypes.bfloat16)

        ext = np.full((BL, S + 2), END, dtype=np.int64)
        ext[:, 0] = START
        ext[:, 1:S + 1] = labels_c
        valid = np.arange(S + 2)[None, :] < (lens_c + 1)[:, None]
        ext = np.where(valid, ext, END)
        CNT = np.zeros((NL, NL), dtype=np.float32)
        pmask = np.arange(S + 1)[None, :] < (lens_c + 1)[:, None]
        np.add.at(CNT, (ext[:, 1:][pmask], ext[:, :-1][pmask]), 1.0)

        CNTL = np.zeros((L,), dtype=np.float32)
        msk = np.arange(S)[None, :] < lens_c[:, None]
        np.add.at(CNTL, labels_c[msk], 1.0)

        SELEND = np.zeros((128, NBLK * BL), dtype=np.float32)
        SELSC = np.zeros((2, NEV * HB), dtype=np.float32)
        SELK = np.zeros((C, BL), dtype=np.float32)
        for bb in range(BL):
            l = int(lens_c[bb])
            cch = 0 if l <= W + G else (l - W - 1) // G
            gg = l - cch * G
            hb, cm = divmod(cch, CH)
            pi = (gg + 1) * CH + cm - CH          # flat pair idx within group
            blk, row = divmod(pi, 128)
            SELEND[row, (hb * BLKH + blk) * BL + bb] = 1.0
            for kev in range(NEV):
                if EVENTS[kev] + DEF <= gg - 1:
                    SELSC[hb, (kev * CH + cm) * BL + bb] = 1.0
            SELK[cch, bb] = 1.0
        SCC = C0 * lens_c.astype(np.float32)

        CPK = np.zeros((128, CPW), dtype=np.float32)
        CPK[0:NL, O_TT:O_TT + NL] = TT
        CPK[0:NL, O_T:O_T + NL] = T
        CPK[0:NL, O_CNT:O_CNT + NL] = -CNT
        CPK[0:L, O_CNTL] = -CNTL
        CPK[0:L, O_BIAS] = b
        CPK[0:L, O_BIASC] = b - C0
        CPK[NLB:NLB + L, O_BIASC] = b - C0
        CPK[:, O_SELEND:O_SELEND + NBLK * BL] = SELEND
        CPK[0:2, O_SELSC:O_SELSC + NEV * HB] = SELSC
        CPK[0:C, O_SELK:O_SELK + BL] = SELK
        CPK[0:C, O_LTRI:O_LTRI + C] = LTRI
        CPK[0:1, O_SCC:O_SCC + BL] = SCC
        CPK[0, O_LT23B + CH:O_LT23B + C] = 1.0
        CPK[1, O_LT23N + CH:O_LT23N + C] = -1.0
        CPK[:, O_ONES] = 1.0

        BSEL = np.zeros((2, NP2), dtype=np.float32)
        BSEL[0, 0:L] = 1.0
        BSEL[1, NLB:NLB + L] = 1.0
        in_maps.append({
            "xT": xT8, "WT": WT8, "OH": OH,
            "UINIT": UINIT, "ELI": ELI, "CPK": CPK,
            "BSEL": BSEL.astype(ml_dtypes.bfloat16),
        })
    return in_maps


_NC_CACHE = []


def kernel(inputs, W, b, transition, lens, labels, _trace=False, _tmpdir=None):
    in_maps = prep_inputs(inputs, W, b, transition, lens, labels)
    if not _NC_CACHE:
        _NC_CACHE.append(build_program())
    nc = _NC_CACHE[0]
    res = run_bass_kernel_spmd(nc, in_maps, list(range(NCORES)),
                               trace=_trace, tmpdir=_tmpdir)
    total = np.float64(0.0)
    for r in res.results:
        total += np.float64(r["loss"][0, 0])
    out = np.float32(total)
    if _trace:
        return out, res
    return out


# revision 5
# speedup vs baseline: 1.1472x; 1.0092x over previous
"""CRF decoder loss kernel for Trainium2 (Bass/Tile), 8-core data parallel.

Chunked warmup-probe forward scan, partition-packed
---------------------------------------------------
The CRF forward recursion u_{t+1} = el_t * (M u_t) (hot domain, M =
exp(T)^T) is a 512-step serial chain whose per-step PE->DVE round-trip
latency dominates. Products of positive matrices contract to rank-1
exponentially fast (Birkhoff), so the time axis is split into C chunks
processed CONCURRENTLY: chunk c's state starts W steps early from a
uniform probe; after W warmup steps its direction matches the true
forward state to ~1e-3 relative (loss tolerance is ~27 nats/seq, so
this is far inside budget). Magnitudes are stitched per chunk boundary
with label-mass-sum ratios (B'_{c-1} - A'_c), prefix-summed by a
lower-triangular matmul (K). Chain length: 512 -> W + G steps.

The C chunks are packed both ways: C/2 chunk-columns in the free dim
AND 2 groups in the partition dim (blockdiag(M, M) stationary,
104x104), halving the per-step DVE time. Per step: one bf16 matmul +
one DVE Hadamard. The END-row trick carries end^T M u_t for every
prefix; per-sequence lengths select norm scores via host-built
one-hots. Per-chunk rescaling (deferred apply) bounds fp16 range.

Emissions: x and W are fp8 e4m3 (W scaled by 16, duplicated in both
partition blocks so logits land in either group's rows); logits
accumulate in f32 PSUM; Act exp (scale=1/16, bias=b-C0 from host)
writes el; warmup-overlap columns are duplicated by the idle Pool
engine (Act handles the two cross-group boundaries). Gold score =
onehot unary + pair counts + bias counts, negated host-side and
accumulated into the same PSUM as the norm terms. DMAs are batched
(packed consts, 3D slice APs) because each DMA instruction occupies
the shared HWDGE descriptor generator ~625ns.
"""

import numpy as np
import ml_dtypes
from contextlib import ExitStack

import concourse.tile as tile
from concourse import bacc
from concourse import mybir
from concourse.bass_utils import run_bass_kernel_spmd

F32 = mybir.dt.float32
FP8 = mybir.dt.float8e4
BF16 = mybir.dt.bfloat16
AF = mybir.ActivationFunctionType
ALU = mybir.AluOpType

B, S, D = 128, 512, 1024
L = 50            # real labels
NL = L + 2        # + START, END
START, END = 50, 51
NCORES = 8
BL = B // NCORES  # 16 sequences per core
KD = D // 128     # contraction chunks for emission matmul

# chunked scan parameters
W = 6                 # warmup steps per probe
C = 46                # time chunks (2 partition groups x 23 free columns)
G = (S - W) // C      # valid steps per chunk (11)
assert W + C * G == S
LSTEPS = W + G        # probe chain length (17)
CH = C // 2           # chunks per partition group (23)
HB = CH * BL          # scan free width (368)
NP2 = 128             # packed partition count (two 52-row blocks)
NLB = 64              # partition base of group B's block
R = 8                 # rescale period
DEF = 3               # deferred rescale apply distance
EVENTS = [g for g in range(R - 1, LSTEPS, R) if g + DEF <= LSTEPS - 1]
NEV = len(EVENTS)     # 1 ([7])
C0 = 7.5              # per-step log damping folded into emission bias
SW = 16.0             # fp8 weight scale

NPAIRH = (LSTEPS + 1) * CH        # (slot,chunk) end-dot pairs per group (414)
BLKH = (NPAIRH + 127) // 128      # extraction blocks per group (4)
NBLK = 2 * BLKH                   # 8
TDMA = 128                        # x DMA slice (timesteps)
NSL = S // TDMA                   # 4 slices
NEM = C + 1                       # emission chunks (46 of G steps + tail W)

# packed-consts column offsets (f32 [128, CPW], one DMA)
O_TT = 0                          # T^T [NL, NL]
O_T = O_TT + NL                   # T   [NL, NL] (gold)
O_CNT = O_T + NL                  # -pair counts [NL, NL]
O_CNTL = O_CNT + NL               # -label counts [L, 1]
O_BIAS = O_CNTL + 1               # raw bias b [L, 1] (gold)
O_BIASC = O_BIAS + 1              # b - C0 at rows 0:50 AND 52:102
O_SELEND = O_BIASC + 1            # [128, NBLK*BL]
O_SELSC = O_SELEND + NBLK * BL    # [2, NEV*HB]
O_SELK = O_SELSC + NEV * HB       # [C, BL]
O_LTRI = O_SELK + BL              # [C, C]
O_SCC = O_LTRI + C                # [1, BL]
O_LT23B = O_SCC + BL              # [2, C]: row0 = +[c>=CH] (boundary fix)
O_LT23N = O_LT23B + C             # [2, C]: row1 = -[c>=CH]
O_ONES = O_LT23N + C              # [128, 1]
CPW = O_ONES + 1


def build_program():
    nc = bacc.Bacc("TRN2", target_bir_lowering=False, debug=False,
                   num_devices=NCORES)

    xT_d = nc.dram_tensor("xT", [KD, 128, S * BL], FP8, kind="ExternalInput")
    WT_d = nc.dram_tensor("WT", [KD, 128, NP2], FP8, kind="ExternalInput")
    OH_d = nc.dram_tensor("OH", [L, S * BL], BF16, kind="ExternalInput")
    UINIT_d = nc.dram_tensor("UINIT", [NP2, HB], BF16, kind="ExternalInput")
    ELI_d = nc.dram_tensor("ELI", [14, LSTEPS * HB], F32, kind="ExternalInput")
    CPK_d = nc.dram_tensor("CPK", [128, CPW], F32, kind="ExternalInput")
    BSEL_d = nc.dram_tensor("BSEL", [2, NP2], BF16, kind="ExternalInput")
    loss_d = nc.dram_tensor("loss", [1, 1], F32, kind="ExternalOutput")

    with tile.TileContext(nc) as tc, ExitStack() as ctx:
        ctx.enter_context(nc.allow_low_precision(reason="bf16 scan state"))
        consts = ctx.enter_context(tc.tile_pool(name="consts", bufs=1))
        xpool = ctx.enter_context(tc.tile_pool(name="xpool", bufs=1))
        smalls = ctx.enter_context(tc.tile_pool(name="smalls", bufs=2))
        lgp = ctx.enter_context(tc.tile_pool(name="lgp", bufs=5, space="PSUM"))
        pp = ctx.enter_context(tc.tile_pool(name="pp", bufs=1, space="PSUM"))
        miscp = ctx.enter_context(tc.tile_pool(name="miscp", bufs=1,
                                               space="PSUM"))

        # ---------------- batched input DMAs + views ----------------
        cpk = consts.tile([128, CPW], F32, name="cpk")
        nc.sync.dma_start(out=cpk[:, :], in_=CPK_d.ap()[:, :])
        ttile = cpk[0:NL, O_TT:O_TT + NL]
        traw = cpk[0:NL, O_T:O_T + NL]
        cnt = cpk[0:NL, O_CNT:O_CNT + NL]
        cntl = cpk[0:L, O_CNTL:O_CNTL + 1]
        braw = cpk[0:L, O_BIAS:O_BIAS + 1]
        selend = cpk[0:128, O_SELEND:O_SELEND + NBLK * BL]
        selsc = cpk[0:2, O_SELSC:O_SELSC + NEV * HB]
        selk = cpk[0:C, O_SELK:O_SELK + BL]
        ltri = cpk[0:C, O_LTRI:O_LTRI + C]
        scc = cpk[0:1, O_SCC:O_SCC + BL]
        lt23b = cpk[0:2, O_LT23B:O_LT23B + C]
        lt23n = cpk[0:2, O_LT23N:O_LT23N + C]
        onesr = cpk[0:128, O_ONES:O_ONES + 1]

        wt = consts.tile([128, KD * NP2], FP8, name="wt")
        nc.sync.dma_start(
            out=wt[:, :].rearrange("p (k l) -> p k l", k=KD, l=NP2),
            in_=WT_d.ap()[0:KD, :, :].rearrange("k p l -> p k l"))

        # x slices with finer granularity at the start so the first
        # emission units start as soon as possible
        SLICES = [(0, 32), (32, 64), (64, 128), (128, 256), (256, 384),
                  (384, 512)]

        def slice_of(t):
            for i, (a, b) in enumerate(SLICES):
                if a <= t < b:
                    return i
            raise AssertionError(t)

        xt_tiles = {}

        def issue_dma(si):
            s0, s1 = SLICES[si]
            cols = (s1 - s0) * BL
            xt = xpool.tile([128, KD * cols], FP8, name="xt", tag=f"xt{si}")
            nc.sync.dma_start(
                out=xt[:, :].rearrange("p (k c) -> p k c", k=KD, c=cols),
                in_=xT_d.ap()[0:KD, :, s0 * BL:s1 * BL]
                .rearrange("k p c -> p k c"))
            xt_tiles[si] = xt

        issue_dma(0)

        oh = consts.tile([L, S * BL], BF16, name="oh")
        nc.scalar.dma_start(out=oh[:, :], in_=OH_d.ap()[:, :])

        # blockdiag stationary: statd[0:52,0:52] = statd[52:104,52:104]
        # = exp(T^T); the second block is filled by an SBUF->SBUF DMA.
        statd = consts.tile([NP2, NP2], BF16, name="statd")
        nc.vector.memset(statd[:, :], 0.0)
        nc.scalar.activation(out=statd[0:NL, 0:NL], in_=ttile, func=AF.Exp)
        nc.scalar.dma_start(out=statd[NLB:NLB + NL, NLB:NLB + NL],
                            in_=statd[0:NL, 0:NL])

        ones = consts.tile([128, 1], F32, name="ones")
        nc.vector.memset(ones[:, :], 1.0)
        # msel: matmul lhsT summing real-label rows per group -> [2, cols]
        msel = consts.tile([NP2, 2], BF16, name="msel")
        nc.vector.memset(msel[:, :], 0.0)
        nc.vector.memset(msel[0:L, 0:1], 1.0)
        nc.vector.memset(msel[NLB:NLB + L, 1:2], 1.0)
        # bsel: broadcast rec rows back to their group's label rows
        bsel = consts.tile([2, NP2], BF16, name="bsel")
        nc.scalar.dma_start(out=bsel[:, :], in_=BSEL_d.ap()[:, :])
        # e2: ones into both END rows (pb=1 there: end-dots unscaled)
        e2 = consts.tile([1, NP2], BF16, name="e2")
        nc.vector.memset(e2[:, :], 0.0)
        nc.vector.memset(e2[0:1, END:END + 1], 1.0)
        nc.vector.memset(e2[0:1, NLB + END:NLB + END + 1], 1.0)
        onesrow = consts.tile([1, HB], BF16, name="onesrow")
        nc.vector.memset(onesrow[:, :], 1.0)

        # ---------------- big state buffers ----------------
        # el_scan: per group, chunk-major [NP2, (cm*LSTEPS + g)*BL + b]
        el_scan = consts.tile([NP2, LSTEPS * HB], F32, name="el_scan")
        # u_slots: slot-major [NP2, (s*CH + cm)*BL + b]
        u_slots = consts.tile([NP2, (LSTEPS + 2) * HB], BF16, name="u_slots")
        scale_row = consts.tile([2, NEV * HB], F32, name="scale_row")
        uacc = consts.tile([L, 32], F32, name="uacc")
        scratch = consts.tile([NL, 2 * G * BL], F32, name="scratch")

        nc.vector.memset(scale_row[:, :], 0.0)
        nc.vector.memset(uacc[:, :], 0.0)
        # ELI rows: START=0, END=1, then 12 zero rows covering the
        # inter-block gap (so the Hadamard never multiplies garbage)
        nc.scalar.dma_start(out=el_scan[START:START + 14, :],
                            in_=ELI_d.ap()[:, :])
        nc.scalar.dma_start(out=el_scan[NLB + START:NLB + START + 14, :],
                            in_=ELI_d.ap()[:, :])
        nc.scalar.dma_start(out=u_slots[:, 0:HB], in_=UINIT_d.ap()[:, :])

        # gold-score constants (negated on host; accumulate into nacc)
        gscr = consts.tile([NL, NL], F32, name="gscr")
        gt1 = consts.tile([NL, 1], F32, name="gt1")
        nc.vector.tensor_mul(gscr[:, :], traw, cnt)
        nc.vector.tensor_reduce(out=gt1[:, :], in_=gscr[:, :],
                                axis=mybir.AxisListType.X, op=ALU.add)
        gt2 = consts.tile([L, 1], F32, name="gt2")
        nc.vector.tensor_mul(gt2[:, :], braw, cntl)

        issue_dma(1)

        # ---------------- emissions ----------------
        def em_chunk(c):
            t0, t1 = c  # unit = absolute time range
            lg = lgp.tile([NP2, 2 * G * BL], F32, name="lg", tag="lg")
            ranges = []
            a = t0
            while a < t1:
                b_end = min(t1, SLICES[slice_of(a)][1])
                ranges.append((a, b_end))
                a = b_end
            for (a, b_end) in ranges:
                si = slice_of(a)
                s0, s1 = SLICES[si]
                xt = xt_tiles[si]
                sb = (s1 - s0) * BL
                co = (a - s0) * BL
                cw = (b_end - a) * BL
                for k in range(KD):
                    nc.tensor.matmul(
                        lg[:, (a - t0) * BL:(a - t0) * BL + cw],
                        lhsT=wt[:, k * NP2:(k + 1) * NP2],
                        rhs=xt[:, k * sb + co:k * sb + co + cw],
                        start=(k == 0), stop=(k == KD - 1))
            return lg

        def em_post(ui, unit, lg):
            t0, t1 = unit
            ncols = (t1 - t0) * BL
            c0 = t0 // G
            nck = (t1 - t0) // G     # 0 (tail), 1, or 2 scan chunks
            if nck >= 1:
                hb = c0 // CH
                cm = c0 % CH
                ro = hb * NLB
                # main exp for 1 or 2 chunks: strided 3D output AP skips
                # each chunk's warmup-tail region in el_scan
                if nck == 2:
                    dst = el_scan[ro:ro + L, cm * LSTEPS * BL:
                                  (cm + 2) * LSTEPS * BL]\
                        .rearrange("p (c x) -> p c x", c=2,
                                   x=LSTEPS * BL)[0:L, 0:2, 0:G * BL]
                    src = lg[ro:ro + L, 0:ncols].rearrange(
                        "p (c x) -> p c x", c=2, x=G * BL)
                else:
                    dst = el_scan[ro:ro + L, cm * LSTEPS * BL:
                                  cm * LSTEPS * BL + G * BL]
                    src = lg[ro:ro + L, 0:ncols]
                nc.scalar.activation(out=dst, in_=src, func=AF.Exp,
                                     bias=cpk[ro:ro + L, O_BIASC:O_BIASC + 1],
                                     scale=1.0 / SW)
                # dup for chunk c0-1 (same group: Pool; boundary: Act re-exp)
                if c0 >= 1:
                    hb2 = (c0 - 1) // CH
                    cm2 = (c0 - 1) % CH
                    ro2 = hb2 * NLB
                    ddst = el_scan[ro2:ro2 + L,
                                   (cm2 * LSTEPS + G) * BL:
                                   (cm2 * LSTEPS + G + W) * BL]
                    if hb2 == hb:
                        dsrc = el_scan[ro:ro + L, cm * LSTEPS * BL:
                                       (cm * LSTEPS) * BL + W * BL]
                        nc.gpsimd.tensor_copy(ddst, dsrc)
                    else:
                        nc.scalar.activation(
                            out=ddst, in_=lg[ro2:ro2 + L, 0:W * BL],
                            func=AF.Exp,
                            bias=cpk[ro2:ro2 + L, O_BIASC:O_BIASC + 1],
                            scale=1.0 / SW)
                if nck == 2:
                    # dup for chunk c0 from chunk c0+1's first W steps
                    ddst = el_scan[ro:ro + L,
                                   (cm * LSTEPS + G) * BL:
                                   (cm * LSTEPS + G + W) * BL]
                    dsrc = el_scan[ro:ro + L, (cm + 1) * LSTEPS * BL:
                                   (cm + 1) * LSTEPS * BL + W * BL]
                    nc.gpsimd.tensor_copy(ddst, dsrc)
            else:
                # tail [C*G, S): dup-style into last chunk of group 1
                ro = NLB
                ddst = el_scan[ro:ro + L,
                               ((CH - 1) * LSTEPS + G) * BL:
                               ((CH - 1) * LSTEPS + G + W) * BL]
                nc.scalar.activation(out=ddst, in_=lg[ro:ro + L, 0:ncols],
                                     func=AF.Exp,
                                     bias=cpk[ro:ro + L, O_BIASC:O_BIASC + 1],
                                     scale=1.0 / SW)
            # gold unary
            nc.vector.tensor_mul(scratch[0:L, 0:ncols], lg[0:L, 0:ncols],
                                 oh[:, t0 * BL:t0 * BL + ncols])
            nc.vector.tensor_reduce(out=uacc[:, ui:ui + 1],
                                    in_=scratch[0:L, 0:ncols],
                                    axis=mybir.AxisListType.X, op=ALU.add)

        # units: pairs of scan chunks within each group + singles + tail
        units = []
        for h in range(2):
            base = h * CH
            cc = base
            while cc < base + CH:
                if cc + 1 < base + CH:
                    units.append((cc * G, (cc + 2) * G))
                    cc += 2
                else:
                    units.append((cc * G, (cc + 1) * G))
                    cc += 1
        units.append((C * G, S))

        for ui, unit in enumerate(units):
            s_hi = slice_of(unit[1] - 1)
            for si in range(len(xt_tiles), min(s_hi + 2, len(SLICES))):
                issue_dma(si)
            lg = em_chunk(unit)
            em_post(ui, unit, lg)

        # gold unary total (PE matmuls accumulate into nacc at the end)
        ur = consts.tile([L, 1], F32, name="ur")
        nc.vector.tensor_reduce(out=ur[:, :], in_=uacc[:, :],
                                axis=mybir.AxisListType.X, op=ALU.add)
        nc.vector.tensor_scalar_mul(ur[:, :], ur[:, :], -1.0 / SW)

        # ---------------- chunked scan ----------------
        endbuf = consts.tile([128, NBLK * BL], BF16, name="endbuf")
        nc.vector.memset(endbuf[:, :], 1.0)
        a_row = consts.tile([2, HB], F32, name="a_row")

        el4 = el_scan[0:NP2, :].rearrange("p (c g b) -> p c g b",
                                          c=CH, g=LSTEPS, b=BL)

        def end_block_dma(hb, q, eng=None):
            # block q of group hb: pairs (slot s>=1, cm), flat = s*CH+cm-CH
            p0 = CH + 128 * q
            p1 = min(p0 + 128, CH + NPAIRH)
            row = hb * NLB + END
            src = u_slots[row:row + 1, p0 * BL:p1 * BL]
            (eng or nc.sync).dma_start(
                out=endbuf[0:p1 - p0,
                           (hb * BLKH + q) * BL:(hb * BLKH + q + 1) * BL],
                in_=src.rearrange("p (q b) -> p q b", q=p1 - p0, b=BL))

        blk_ready = {}
        for q in range(BLKH - 1):
            blk_ready.setdefault((CH + 128 * (q + 1) - 1) // CH - 1,
                                 []).append(q)

        pend = {}
        for g in range(LSTEPS):
            p = pp.tile([NP2, HB], F32, name="p", tag="p")
            nc.tensor.matmul(p[:, :], lhsT=statd[:, :],
                             rhs=u_slots[:, g * HB:(g + 1) * HB],
                             start=True, stop=True)
            out3 = u_slots[:, (g + 1) * HB:(g + 2) * HB].rearrange(
                "p (c b) -> p c b", c=CH, b=BL)
            p3 = p[:, :].rearrange("p (c b) -> p c b", c=CH, b=BL)
            nc.vector.tensor_mul(out3, p3, el4[0:NP2, 0:CH, g, 0:BL])

            if g in pend:
                pb = pend.pop(g)
                nc.vector.tensor_mul(
                    u_slots[:, (g + 1) * HB:(g + 2) * HB],
                    u_slots[:, (g + 1) * HB:(g + 2) * HB],
                    pb[:, :])

            if g in EVENTS:
                kev = EVENTS.index(g)
                ps = miscp.tile([2, HB], F32, name="ps", tag="m1")
                nc.tensor.matmul(ps[:, :], lhsT=msel[:, :],
                                 rhs=u_slots[:, (g + 1) * HB:(g + 2) * HB],
                                 start=True, stop=True)
                nc.scalar.activation(
                    out=scale_row[:, kev * HB:(kev + 1) * HB],
                    in_=ps[:, :], func=AF.Ln)
                rec = smalls.tile([2, HB], BF16, name="rec", tag="rec")
                nc.vector.reciprocal(rec[:, :], ps[:, :])
                pb = miscp.tile([NP2, HB], F32, name="pb", tag="m2")
                nc.tensor.matmul(pb[:, :], lhsT=bsel[:, :], rhs=rec[:, :],
                                 start=True, stop=False)
                nc.tensor.matmul(pb[:, :], lhsT=e2[:, :], rhs=onesrow[:, :],
                                 start=False, stop=True)
                pend[g + DEF] = pb

            if g == W - 2:
                # boundary mass A from slot W-1 (5 warmup steps: direction
                # already converged to ~5e-3, far inside tolerance)
                bw = miscp.tile([2, HB], F32, name="bw", tag="m1")
                nc.tensor.matmul(bw[:, :], lhsT=msel[:, :],
                                 rhs=u_slots[:, (W - 1) * HB:W * HB],
                                 start=True, stop=True)
                nc.scalar.activation(out=a_row[:, :], in_=bw[:, :], func=AF.Ln)

            if g == EVENTS[0] + 1:
                # scale-log selection (scale_row complete after last event)
                scm = consts.tile([2, NEV * HB], F32, name="scm")
                nc.vector.tensor_mul(scm[:, :], scale_row[:, :], selsc)
                scred = consts.tile([2, BL], F32, name="scred")
                nc.vector.tensor_reduce(
                    out=scred[:, :],
                    in_=scm[:, :].rearrange("p (c b) -> p b c",
                                            c=NEV * CH, b=BL),
                    axis=mybir.AxisListType.X, op=ALU.add)

            if g == LSTEPS - 2:
                # stitch from slot LSTEPS-1 (same absolute boundary as the
                # A-side slot W-1), overlapping the scan tail
                be = miscp.tile([2, HB], F32, name="be", tag="m1")
                nc.tensor.matmul(be[:, :], lhsT=msel[:, :],
                                 rhs=u_slots[:, (LSTEPS - 1) * HB:LSTEPS * HB],
                                 start=True, stop=True)
                b_row = consts.tile([2, HB], F32, name="b_row")
                nc.scalar.activation(out=b_row[:, :], in_=be[:, :], func=AF.Ln)
                for kev in range(NEV):
                    nc.vector.tensor_add(b_row[:, :], b_row[:, :],
                                         scale_row[:, kev * HB:(kev + 1) * HB])
                # D in [2, HB] form; the cross-group boundary element is
                # patched into kacc by a rank-1 correction matmul (lt23)
                d2 = consts.tile([2, HB], F32, name="d2")
                nc.vector.memset(d2[:, :], 0.0)
                nc.vector.tensor_sub(d2[0:2, BL:HB], b_row[0:2, 0:HB - BL],
                                     a_row[0:2, BL:HB])
                dm = consts.tile([C, BL], F32, name="dm")
                nc.sync.dma_start(out=dm[:, :],
                                  in_=d2[:, :].rearrange(
                                      "p (c b) -> p c b", c=CH, b=BL))
                kacc = miscp.tile([C, BL], F32, name="kacc", tag="m2")
                nc.tensor.matmul(kacc[:, :], lhsT=ltri, rhs=dm[:, :],
                                 start=True, stop=False)
                nc.tensor.matmul(kacc[:, :], lhsT=lt23b,
                                 rhs=b_row[0:2, HB - BL:HB],
                                 start=False, stop=False)
                nc.tensor.matmul(kacc[:, :], lhsT=lt23n,
                                 rhs=a_row[0:2, 0:BL],
                                 start=False, stop=True)
                kmask = consts.tile([C, BL], F32, name="kmask")
                nc.vector.tensor_mul(kmask[:, :], kacc[:, :], selk)

            if g in blk_ready:
                for q in blk_ready[g]:
                    end_block_dma(0, q)
                    end_block_dma(1, q)

        # early Ln for extraction blocks that landed during the scan
        # (0..BLKH-2 of each group); the pf-dependent blocks follow later
        endlog = consts.tile([128, NBLK * BL], F32, name="endlog")
        eb = BLKH - 1
        nc.vector.tensor_scalar_max(endbuf[:, 0:eb * BL],
                                    endbuf[:, 0:eb * BL], 1e-38)
        nc.vector.tensor_scalar_max(endbuf[:, BLKH * BL:(BLKH + eb) * BL],
                                    endbuf[:, BLKH * BL:(BLKH + eb) * BL],
                                    1e-38)
        nc.scalar.activation(out=endlog[:, 0:eb * BL],
                             in_=endbuf[:, 0:eb * BL], func=AF.Ln)
        nc.scalar.activation(out=endlog[:, BLKH * BL:(BLKH + eb) * BL],
                             in_=endbuf[:, BLKH * BL:(BLKH + eb) * BL],
                             func=AF.Ln)

        # final end-dots for states at slot LSTEPS
        pf = pp.tile([NP2, HB], F32, name="pf", tag="p")
        nc.tensor.matmul(pf[:, :], lhsT=statd[:, :],
                         rhs=u_slots[:, LSTEPS * HB:(LSTEPS + 1) * HB],
                         start=True, stop=True)
        # only the END rows of the final slot matter; copy the 32-quad
        # containing each block's END row (offset starts limited to 32 rows)
        nc.scalar.copy(u_slots[32:64, (LSTEPS + 1) * HB:(LSTEPS + 2) * HB],
                       pf[32:64, :])
        nc.vector.tensor_copy(
            u_slots[96:128, (LSTEPS + 1) * HB:(LSTEPS + 2) * HB],
            pf[96:128, :])
        end_block_dma(0, BLKH - 1, eng=nc.sync)
        end_block_dma(1, BLKH - 1, eng=nc.scalar)

        # ---------------- norm score selection ----------------
        for blk in (BLKH - 1, NBLK - 1):
            nc.vector.tensor_scalar_max(endbuf[:, blk * BL:(blk + 1) * BL],
                                        endbuf[:, blk * BL:(blk + 1) * BL],
                                        1e-38)
            nc.scalar.activation(out=endlog[:, blk * BL:(blk + 1) * BL],
                                 in_=endbuf[:, blk * BL:(blk + 1) * BL],
                                 func=AF.Ln)
        nc.vector.tensor_mul(endlog[:, :], endlog[:, :], selend)
        esum = consts.tile([128, BL], F32, name="esum")
        nc.vector.tensor_reduce(
            out=esum[:, :],
            in_=endlog[:, :].rearrange("p (blk b) -> p b blk",
                                       blk=NBLK, b=BL),
            axis=mybir.AxisListType.X, op=ALU.add)

        nacc = miscp.tile([1, BL], F32, name="nacc", tag="m1")
        nc.tensor.matmul(nacc[:, :], lhsT=onesr, rhs=esum[:, :],
                         start=True, stop=False)
        nc.tensor.matmul(nacc[:, :], lhsT=onesr[0:2, :], rhs=scred[:, :],
                         start=False, stop=False)
        nc.tensor.matmul(nacc[:, :], lhsT=onesr[0:C, :], rhs=kmask[:, :],
                         start=False, stop=False)
        nc.tensor.matmul(nacc[:, :], lhsT=onesr[0:1, :], rhs=scc,
                         start=False, stop=False)
        # negated gold pieces into column 0
        nc.tensor.matmul(nacc[:, 0:1], lhsT=ones[0:NL, :], rhs=gt1[:, :],
                         start=False, stop=False)
        nc.tensor.matmul(nacc[:, 0:1], lhsT=ones[0:L, :], rhs=gt2[:, :],
                         start=False, stop=False)
        nc.tensor.matmul(nacc[:, 0:1], lhsT=ones[0:L, :], rhs=ur[:, :],
                         start=False, stop=True)

        # loss = sum_b (norm - gold)
        dbgt = smalls.tile([1, 1], F32, name="dbgt", tag="dbgt")
        nc.vector.tensor_reduce(out=dbgt[0:1, 0:1], in_=nacc[:, :],
                                axis=mybir.AxisListType.X, op=ALU.add)
        nc.sync.dma_start(out=loss_d.ap()[:, :], in_=dbgt[:, :])

    nc.compile()
    return nc


def prep_inputs(inputs, W_in, b, transition, lens, labels):
    """Host-side sharding + index preprocessing. Returns per-core input maps."""
    x = np.ascontiguousarray(np.asarray(inputs, dtype=np.float32))
    Wm = np.asarray(W_in, dtype=np.float32)
    b = np.asarray(b, dtype=np.float32)
    T = np.asarray(transition, dtype=np.float32)
    lens = np.asarray(lens).astype(np.int64)
    labels = np.asarray(labels).astype(np.int64)

    # W duplicated in both partition blocks, scaled for fp8
    WT = np.zeros((KD, 128, NP2), dtype=np.float32)
    Wk = np.ascontiguousarray((Wm * SW).T).reshape(KD, 128, L)
    WT[:, :, 0:L] = Wk
    WT[:, :, NLB:NLB + L] = Wk
    WT8 = WT.astype(ml_dtypes.float8_e4m3)
    TT = np.ascontiguousarray(T.T)

    xt_all = np.ascontiguousarray(np.transpose(x, (2, 1, 0)))  # (D, S, B)

    ELI = np.zeros((14, LSTEPS * HB), dtype=np.float32)
    ELI[1, :] = 1.0
    UINIT = np.zeros((NP2, HB), dtype=np.float32)
    UINIT[START, 0:BL] = 1.0                       # chunk 0: true init
    UINIT[:L, BL:] = 1.0 / L                       # group A probes
    UINIT[NLB:NLB + L, :] = 1.0 / L                # group B probes
    UINIT = UINIT.astype(ml_dtypes.bfloat16)
    LTRI = np.zeros((C, C), dtype=np.float32)
    for cc in range(C):
        LTRI[:cc + 1, cc] = 1.0

    in_maps = []
    for core in range(NCORES):
        bs = slice(core * BL, (core + 1) * BL)
        lens_c = lens[bs]
        labels_c = labels[bs]

        xT = np.ascontiguousarray(xt_all[:, :, bs]).reshape(KD, 128, S * BL)
        xT8 = xT.astype(ml_dtypes.float8_e4m3)

        mask = np.arange(S)[:, None] < lens_c[None, :]
        lab_t = labels_c.T
        OH = (lab_t[None, :, :] == np.arange(L)[:, None, None]) & mask[None]
        OH = np.ascontiguousarray(
            OH.reshape(L, S * BL)).astype(ml_dtypes.bfloat16)

        ext = np.full((BL, S + 2), END, dtype=np.int64)
        ext[:, 0] = START
        ext[:, 1:S + 1] = labels_c
        valid = np.arange(S + 2)[None, :] < (lens_c + 1)[:, None]
        ext = np.where(valid, ext, END)
        CNT = np.zeros((NL, NL), dtype=np.float32)
        pmask = np.arange(S + 1)[None, :] < (lens_c + 1)[:, None]
        np.add.at(CNT, (ext[:, 1:][pmask], ext[:, :-1][pmask]), 1.0)

        CNTL = np.zeros((L,), dtype=np.float32)
        msk = np.arange(S)[None, :] < lens_c[:, None]
        np.add.at(CNTL, labels_c[msk], 1.0)

        SELEND = np.zeros((128, NBLK * BL), dtype=np.float32)
        SELSC = np.zeros((2, NEV * HB), dtype=np.float32)
        SELK = np.zeros((C, BL), dtype=np.float32)
        for bb in range(BL):
            l = int(lens_c[bb])
            cch = 0 if l <= W + G else (l - W - 1) // G
            gg = l - cch * G
            hb, cm = divmod(cch, CH)
            pi = (gg + 1) * CH + cm - CH          # flat pair idx within group
            blk, row = divmod(pi, 128)
            SELEND[row, (hb * BLKH + blk) * BL + bb] = 1.0
            for kev in range(NEV):
                if EVENTS[kev] + DEF <= gg - 1:
                    SELSC[hb, (kev * CH + cm) * BL + bb] = 1.0
            SELK[cch, bb] = 1.0
        SCC = C0 * lens_c.astype(np.float32)

        CPK = np.zeros((128, CPW), dtype=np.float32)
        CPK[0:NL, O_TT:O_TT + NL] = TT
        CPK[0:NL, O_T:O_T + NL] = T
        CPK[0:NL, O_CNT:O_CNT + NL] = -CNT
        CPK[0:L, O_CNTL] = -CNTL
        CPK[0:L, O_BIAS] = b
        CPK[0:L, O_BIASC] = b - C0
        CPK[NLB:NLB + L, O_BIASC] = b - C0
        CPK[:, O_SELEND:O_SELEND + NBLK * BL] = SELEND
        CPK[0:2, O_SELSC:O_SELSC + NEV * HB] = SELSC
        CPK[0:C, O_SELK:O_SELK + BL] = SELK
        CPK[0:C, O_LTRI:O_LTRI + C] = LTRI
        CPK[0:1, O_SCC:O_SCC + BL] = SCC
        CPK[0, O_LT23B + CH:O_LT23B + C] = 1.0
        CPK[1, O_LT23N + CH:O_LT23N + C] = -1.0
        CPK[:, O_ONES] = 1.0

        BSEL = np.zeros((2, NP2), dtype=np.float32)
        BSEL[0, 0:L] = 1.0
        BSEL[1, NLB:NLB + L] = 1.0
        in_maps.append({
            "xT": xT8, "WT": WT8, "OH": OH,
            "UINIT": UINIT, "ELI": ELI, "CPK": CPK,
            "BSEL": BSEL.astype(ml_dtypes.bfloat16),
        })
    return in_maps


_NC_CACHE = []


def kernel(inputs, W, b, transition, lens, labels, _trace=False, _tmpdir=None):
    in_maps = prep_inputs(inputs, W, b, transition, lens, labels)
    if not _NC_CACHE:
        _NC_CACHE.append(build_program())
    nc = _NC_CACHE[0]
    res = run_bass_kernel_spmd(nc, in_maps, list(range(NCORES)),
                               trace=_trace, tmpdir=_tmpdir)
    total = np.float64(0.0)
    for r in res.results:
        total += np.float64(r["loss"][0, 0])
    out = np.float32(total)
    if _trace:
        return out, res
    return out


# revision 6
# speedup vs baseline: 1.1563x; 1.0079x over previous
"""CRF decoder loss kernel for Trainium2 (Bass/Tile), 8-core data parallel.

Chunked warmup-probe forward scan, partition-packed
---------------------------------------------------
The CRF forward recursion u_{t+1} = el_t * (M u_t) (hot domain, M =
exp(T)^T) is a 512-step serial chain whose per-step PE->DVE round-trip
latency dominates. Products of positive matrices contract to rank-1
exponentially fast (Birkhoff), so the time axis is split into C chunks
processed CONCURRENTLY: chunk c's state starts W steps early from a
uniform probe; after W warmup steps its direction matches the true
forward state to ~1e-3 relative (loss tolerance is ~27 nats/seq, so
this is far inside budget). Magnitudes are stitched per chunk boundary
with label-mass-sum ratios (B'_{c-1} - A'_c), prefix-summed by a
lower-triangular matmul (K). Chain length: 512 -> W + G steps.

The C chunks are packed both ways: C/2 chunk-columns in the free dim
AND 2 groups in the partition dim (blockdiag(M, M) stationary,
104x104), halving the per-step DVE time. Per step: one bf16 matmul +
one DVE Hadamard. The END-row trick carries end^T M u_t for every
prefix; per-sequence lengths select norm scores via host-built
one-hots. Per-chunk rescaling (deferred apply) bounds fp16 range.

Emissions: x and W are fp8 e4m3 (W scaled by 16, duplicated in both
partition blocks so logits land in either group's rows); logits
accumulate in f32 PSUM; Act exp (scale=1/16, bias=b-C0 from host)
writes el; warmup-overlap columns are duplicated by the idle Pool
engine (Act handles the two cross-group boundaries). Gold score =
onehot unary + pair counts + bias counts, negated host-side and
accumulated into the same PSUM as the norm terms. DMAs are batched
(packed consts, 3D slice APs) because each DMA instruction occupies
the shared HWDGE descriptor generator ~625ns.
"""

import numpy as np
import ml_dtypes
from contextlib import ExitStack

import concourse.tile as tile
from concourse import bacc
from concourse import mybir
from concourse.bass_utils import run_bass_kernel_spmd

F32 = mybir.dt.float32
FP8 = mybir.dt.float8e4
BF16 = mybir.dt.bfloat16
AF = mybir.ActivationFunctionType
ALU = mybir.AluOpType

B, S, D = 128, 512, 1024
L = 50            # real labels
NL = L + 2        # + START, END
START, END = 50, 51
NCORES = 8
BL = B // NCORES  # 16 sequences per core
KD = D // 128     # contraction chunks for emission matmul

# chunked scan parameters
W = 6                 # warmup steps per probe
C = 46                # time chunks (2 partition groups x 23 free columns)
G = (S - W) // C      # valid steps per chunk (11)
assert W + C * G == S
LSTEPS = W + G        # probe chain length (17)
CH = C // 2           # chunks per partition group (23)
HB = CH * BL          # scan free width (368)
NP2 = 128             # packed partition count (two 52-row blocks)
NLB = 64              # partition base of group B's block
R = 8                 # rescale period
DEF = 3               # deferred rescale apply distance
EVENTS = [g for g in range(R - 1, LSTEPS, R) if g + DEF <= LSTEPS - 1]
NEV = len(EVENTS)     # 1 ([7])
C0 = 7.5              # per-step log damping folded into emission bias
SW = 16.0             # fp8 weight scale

NPAIRH = (LSTEPS + 1) * CH        # (slot,chunk) end-dot pairs per group (414)
BLKH = (NPAIRH + 127) // 128      # extraction blocks per group (4)
NBLK = 2 * BLKH                   # 8
TDMA = 128                        # x DMA slice (timesteps)
NSL = S // TDMA                   # 4 slices
NEM = C + 1                       # emission chunks (46 of G steps + tail W)

# packed-consts column offsets (f32 [128, CPW], one DMA)
O_TT = 0                          # T^T [NL, NL]
O_T = O_TT + NL                   # T   [NL, NL] (gold)
O_CNT = O_T + NL                  # -pair counts [NL, NL]
O_CNTL = O_CNT + NL               # -label counts [L, 1]
O_BIAS = O_CNTL + 1               # raw bias b [L, 1] (gold)
O_BIASC = O_BIAS + 1              # b - C0 at rows 0:50 AND 52:102
O_SELEND = O_BIASC + 1            # [128, NBLK*BL]
O_SELSC = O_SELEND + NBLK * BL    # [2, NEV*HB]
O_SELK = O_SELSC + NEV * HB       # [C, BL]
O_LTRI = O_SELK + BL              # [C, C]
O_SCC = O_LTRI + C                # [1, BL]
O_LT23B = O_SCC + BL              # [2, C]: row0 = +[c>=CH] (boundary fix)
O_LT23N = O_LT23B + C             # [2, C]: row1 = -[c>=CH]
O_ONES = O_LT23N + C              # [128, 1]
CPW = O_ONES + 1


def build_program():
    nc = bacc.Bacc("TRN2", target_bir_lowering=False, debug=False,
                   num_devices=NCORES)

    xT_d = nc.dram_tensor("xT", [KD, 128, S * BL], FP8, kind="ExternalInput")
    WT_d = nc.dram_tensor("WT", [KD, 128, NP2], FP8, kind="ExternalInput")
    OH_d = nc.dram_tensor("OH", [L, S * BL], BF16, kind="ExternalInput")
    UINIT_d = nc.dram_tensor("UINIT", [NP2, HB], BF16, kind="ExternalInput")
    ELI_d = nc.dram_tensor("ELI", [14, LSTEPS * HB], F32, kind="ExternalInput")
    CPK_d = nc.dram_tensor("CPK", [128, CPW], F32, kind="ExternalInput")
    BSEL_d = nc.dram_tensor("BSEL", [2, NP2], BF16, kind="ExternalInput")
    loss_d = nc.dram_tensor("loss", [1, 1], F32, kind="ExternalOutput")

    with tile.TileContext(nc) as tc, ExitStack() as ctx:
        ctx.enter_context(nc.allow_low_precision(reason="bf16 scan state"))
        consts = ctx.enter_context(tc.tile_pool(name="consts", bufs=1))
        xpool = ctx.enter_context(tc.tile_pool(name="xpool", bufs=1))
        smalls = ctx.enter_context(tc.tile_pool(name="smalls", bufs=2))
        lgp = ctx.enter_context(tc.tile_pool(name="lgp", bufs=5, space="PSUM"))
        pp = ctx.enter_context(tc.tile_pool(name="pp", bufs=1, space="PSUM"))
        miscp = ctx.enter_context(tc.tile_pool(name="miscp", bufs=1,
                                               space="PSUM"))

        # ---------------- batched input DMAs + views ----------------
        cpk = consts.tile([128, CPW], F32, name="cpk")
        nc.sync.dma_start(out=cpk[:, :], in_=CPK_d.ap()[:, :])
        ttile = cpk[0:NL, O_TT:O_TT + NL]
        traw = cpk[0:NL, O_T:O_T + NL]
        cnt = cpk[0:NL, O_CNT:O_CNT + NL]
        cntl = cpk[0:L, O_CNTL:O_CNTL + 1]
        braw = cpk[0:L, O_BIAS:O_BIAS + 1]
        selend = cpk[0:128, O_SELEND:O_SELEND + NBLK * BL]
        selsc = cpk[0:2, O_SELSC:O_SELSC + NEV * HB]
        selk = cpk[0:C, O_SELK:O_SELK + BL]
        ltri = cpk[0:C, O_LTRI:O_LTRI + C]
        scc = cpk[0:1, O_SCC:O_SCC + BL]
        lt23b = cpk[0:2, O_LT23B:O_LT23B + C]
        lt23n = cpk[0:2, O_LT23N:O_LT23N + C]
        onesr = cpk[0:128, O_ONES:O_ONES + 1]

        wt = consts.tile([128, KD * NP2], FP8, name="wt")
        nc.sync.dma_start(
            out=wt[:, :].rearrange("p (k l) -> p k l", k=KD, l=NP2),
            in_=WT_d.ap()[0:KD, :, :].rearrange("k p l -> p k l"))

        # x slices with finer granularity at the start so the first
        # emission units start as soon as possible
        SLICES = [(0, 32), (32, 64), (64, 128), (128, 256), (256, 384),
                  (384, 512)]

        def slice_of(t):
            for i, (a, b) in enumerate(SLICES):
                if a <= t < b:
                    return i
            raise AssertionError(t)

        xt_tiles = {}

        def issue_dma(si):
            s0, s1 = SLICES[si]
            cols = (s1 - s0) * BL
            xt = xpool.tile([128, KD * cols], FP8, name="xt", tag=f"xt{si}")
            nc.sync.dma_start(
                out=xt[:, :].rearrange("p (k c) -> p k c", k=KD, c=cols),
                in_=xT_d.ap()[0:KD, :, s0 * BL:s1 * BL]
                .rearrange("k p c -> p k c"))
            xt_tiles[si] = xt

        issue_dma(0)

        oh = consts.tile([L, S * BL], BF16, name="oh")
        nc.scalar.dma_start(out=oh[:, :], in_=OH_d.ap()[:, :])

        # blockdiag stationary: statd[0:52,0:52] = statd[52:104,52:104]
        # = exp(T^T); the second block is filled by an SBUF->SBUF DMA.
        statd = consts.tile([NP2, NP2], BF16, name="statd")
        nc.vector.memset(statd[:, :], 0.0)
        nc.scalar.activation(out=statd[0:NL, 0:NL], in_=ttile, func=AF.Exp)
        nc.scalar.dma_start(out=statd[NLB:NLB + NL, NLB:NLB + NL],
                            in_=statd[0:NL, 0:NL])

        ones = consts.tile([128, 1], F32, name="ones")
        nc.vector.memset(ones[:, :], 1.0)
        # msel: matmul lhsT summing real-label rows per group -> [2, cols]
        msel = consts.tile([NP2, 2], BF16, name="msel")
        nc.vector.memset(msel[:, :], 0.0)
        nc.vector.memset(msel[0:L, 0:1], 1.0)
        nc.vector.memset(msel[NLB:NLB + L, 1:2], 1.0)
        # bsel: broadcast rec rows back to their group's label rows
        bsel = consts.tile([2, NP2], BF16, name="bsel")
        nc.scalar.dma_start(out=bsel[:, :], in_=BSEL_d.ap()[:, :])
        # e2: ones into both END rows (pb=1 there: end-dots unscaled)
        e2 = consts.tile([1, NP2], BF16, name="e2")
        nc.vector.memset(e2[:, :], 0.0)
        nc.vector.memset(e2[0:1, END:END + 1], 1.0)
        nc.vector.memset(e2[0:1, NLB + END:NLB + END + 1], 1.0)
        onesrow = consts.tile([1, HB], BF16, name="onesrow")
        nc.vector.memset(onesrow[:, :], 1.0)

        # ---------------- big state buffers ----------------
        # el_scan: per group, chunk-major [NP2, (cm*LSTEPS + g)*BL + b]
        el_scan = consts.tile([NP2, LSTEPS * HB], F32, name="el_scan")
        # u_slots: slot-major [NP2, (s*CH + cm)*BL + b]
        u_slots = consts.tile([NP2, (LSTEPS + 2) * HB], BF16, name="u_slots")
        scale_row = consts.tile([2, NEV * HB], F32, name="scale_row")
        uacc = consts.tile([L, 32], F32, name="uacc")
        scratch = consts.tile([NL, 2 * G * BL], F32, name="scratch")

        nc.vector.memset(scale_row[:, :], 0.0)
        nc.vector.memset(uacc[:, :], 0.0)
        # ELI rows: START=0, END=1, then 12 zero rows covering the
        # inter-block gap (so the Hadamard never multiplies garbage)
        nc.scalar.dma_start(out=el_scan[START:START + 14, :],
                            in_=ELI_d.ap()[:, :])
        nc.scalar.dma_start(out=el_scan[NLB + START:NLB + START + 14, :],
                            in_=ELI_d.ap()[:, :])
        nc.scalar.dma_start(out=u_slots[:, 0:HB], in_=UINIT_d.ap()[:, :])

        # gold-score constants (negated on host; accumulate into nacc)
        gscr = consts.tile([NL, NL], F32, name="gscr")
        gt1 = consts.tile([NL, 1], F32, name="gt1")
        nc.vector.tensor_mul(gscr[:, :], traw, cnt)
        nc.vector.tensor_reduce(out=gt1[:, :], in_=gscr[:, :],
                                axis=mybir.AxisListType.X, op=ALU.add)
        gt2 = consts.tile([L, 1], F32, name="gt2")
        nc.vector.tensor_mul(gt2[:, :], braw, cntl)

        issue_dma(1)

        # ---------------- emissions ----------------
        def em_chunk(c):
            t0, t1 = c  # unit = absolute time range
            lg = lgp.tile([NP2, 2 * G * BL], F32, name="lg", tag="lg")
            ranges = []
            a = t0
            while a < t1:
                b_end = min(t1, SLICES[slice_of(a)][1])
                ranges.append((a, b_end))
                a = b_end
            for (a, b_end) in ranges:
                si = slice_of(a)
                s0, s1 = SLICES[si]
                xt = xt_tiles[si]
                sb = (s1 - s0) * BL
                co = (a - s0) * BL
                cw = (b_end - a) * BL
                for k in range(KD):
                    nc.tensor.matmul(
                        lg[:, (a - t0) * BL:(a - t0) * BL + cw],
                        lhsT=wt[:, k * NP2:(k + 1) * NP2],
                        rhs=xt[:, k * sb + co:k * sb + co + cw],
                        start=(k == 0), stop=(k == KD - 1))
            return lg

        def em_post(ui, unit, lg):
            t0, t1 = unit
            ncols = (t1 - t0) * BL
            c0 = t0 // G
            nck = (t1 - t0) // G     # 0 (tail), 1, or 2 scan chunks
            if nck >= 1:
                hb = c0 // CH
                cm = c0 % CH
                ro = hb * NLB
                # main exp for 1 or 2 chunks: strided 3D output AP skips
                # each chunk's warmup-tail region in el_scan
                if nck == 2:
                    dst = el_scan[ro:ro + L, cm * LSTEPS * BL:
                                  (cm + 2) * LSTEPS * BL]\
                        .rearrange("p (c x) -> p c x", c=2,
                                   x=LSTEPS * BL)[0:L, 0:2, 0:G * BL]
                    src = lg[ro:ro + L, 0:ncols].rearrange(
                        "p (c x) -> p c x", c=2, x=G * BL)
                else:
                    dst = el_scan[ro:ro + L, cm * LSTEPS * BL:
                                  cm * LSTEPS * BL + G * BL]
                    src = lg[ro:ro + L, 0:ncols]
                nc.scalar.activation(out=dst, in_=src, func=AF.Exp,
                                     bias=cpk[ro:ro + L, O_BIASC:O_BIASC + 1],
                                     scale=1.0 / SW)
                # dup for chunk c0-1 (same group: Pool; boundary: Act re-exp)
                if c0 >= 1:
                    hb2 = (c0 - 1) // CH
                    cm2 = (c0 - 1) % CH
                    ro2 = hb2 * NLB
                    ddst = el_scan[ro2:ro2 + L,
                                   (cm2 * LSTEPS + G) * BL:
                                   (cm2 * LSTEPS + G + W) * BL]
                    if hb2 == hb:
                        dsrc = el_scan[ro:ro + L, cm * LSTEPS * BL:
                                       (cm * LSTEPS) * BL + W * BL]
                        nc.gpsimd.tensor_copy(ddst, dsrc)
                    else:
                        nc.scalar.activation(
                            out=ddst, in_=lg[ro2:ro2 + L, 0:W * BL],
                            func=AF.Exp,
                            bias=cpk[ro2:ro2 + L, O_BIASC:O_BIASC + 1],
                            scale=1.0 / SW)
                if nck == 2:
                    # dup for chunk c0 from chunk c0+1's first W steps
                    ddst = el_scan[ro:ro + L,
                                   (cm * LSTEPS + G) * BL:
                                   (cm * LSTEPS + G + W) * BL]
                    dsrc = el_scan[ro:ro + L, (cm + 1) * LSTEPS * BL:
                                   (cm + 1) * LSTEPS * BL + W * BL]
                    nc.gpsimd.tensor_copy(ddst, dsrc)
            else:
                # tail [C*G, S): dup-style into last chunk of group 1
                ro = NLB
                ddst = el_scan[ro:ro + L,
                               ((CH - 1) * LSTEPS + G) * BL:
                               ((CH - 1) * LSTEPS + G + W) * BL]
                nc.scalar.activation(out=ddst, in_=lg[ro:ro + L, 0:ncols],
                                     func=AF.Exp,
                                     bias=cpk[ro:ro + L, O_BIASC:O_BIASC + 1],
                                     scale=1.0 / SW)
            # gold unary
            nc.vector.tensor_mul(scratch[0:L, 0:ncols], lg[0:L, 0:ncols],
                                 oh[:, t0 * BL:t0 * BL + ncols])
            nc.vector.tensor_reduce(out=uacc[:, ui:ui + 1],
                                    in_=scratch[0:L, 0:ncols],
                                    axis=mybir.AxisListType.X, op=ALU.add)

        # units: pairs of scan chunks within each group + singles + tail
        units = []
        for h in range(2):
            base = h * CH
            cc = base
            while cc < base + CH:
                if cc + 1 < base + CH:
                    units.append((cc * G, (cc + 2) * G))
                    cc += 2
                else:
                    units.append((cc * G, (cc + 1) * G))
                    cc += 1
        units.append((C * G, S))

        for ui, unit in enumerate(units):
            s_hi = slice_of(unit[1] - 1)
            for si in range(len(xt_tiles), min(s_hi + 2, len(SLICES))):
                issue_dma(si)
            lg = em_chunk(unit)
            em_post(ui, unit, lg)

        # gold unary total (PE matmuls accumulate into nacc at the end)
        ur = consts.tile([L, 1], F32, name="ur")
        nc.vector.tensor_reduce(out=ur[:, :], in_=uacc[:, :],
                                axis=mybir.AxisListType.X, op=ALU.add)
        nc.vector.tensor_scalar_mul(ur[:, :], ur[:, :], -1.0 / SW)

        # ---------------- chunked scan ----------------
        endbuf = consts.tile([128, NBLK * BL], BF16, name="endbuf")
        nc.vector.memset(endbuf[:, :], 1.0)
        a_row = consts.tile([2, HB], F32, name="a_row")

        el4 = el_scan[0:NP2, :].rearrange("p (c g b) -> p c g b",
                                          c=CH, g=LSTEPS, b=BL)

        def end_block_dma(hb, q, eng=None):
            # block q of group hb: pairs (slot s>=1, cm), flat = s*CH+cm-CH
            p0 = CH + 128 * q
            p1 = min(p0 + 128, CH + NPAIRH)
            row = hb * NLB + END
            src = u_slots[row:row + 1, p0 * BL:p1 * BL]
            (eng or nc.sync).dma_start(
                out=endbuf[0:p1 - p0,
                           (hb * BLKH + q) * BL:(hb * BLKH + q + 1) * BL],
                in_=src.rearrange("p (q b) -> p q b", q=p1 - p0, b=BL))

        blk_ready = {}
        for q in range(BLKH - 1):
            blk_ready.setdefault((CH + 128 * (q + 1) - 1) // CH - 1,
                                 []).append(q)

        pend = {}
        for g in range(LSTEPS):
            p = pp.tile([NP2, HB], F32, name="p", tag="p")
            nc.tensor.matmul(p[:, :], lhsT=statd[:, :],
                             rhs=u_slots[:, g * HB:(g + 1) * HB],
                             start=True, stop=True)
            out3 = u_slots[:, (g + 1) * HB:(g + 2) * HB].rearrange(
                "p (c b) -> p c b", c=CH, b=BL)
            p3 = p[:, :].rearrange("p (c b) -> p c b", c=CH, b=BL)
            nc.vector.tensor_mul(out3, p3, el4[0:NP2, 0:CH, g, 0:BL])

            if g in pend:
                pb = pend.pop(g)
                nc.vector.tensor_mul(
                    u_slots[:, (g + 1) * HB:(g + 2) * HB],
                    u_slots[:, (g + 1) * HB:(g + 2) * HB],
                    pb[:, :])

            if g in EVENTS:
                kev = EVENTS.index(g)
                ps = miscp.tile([2, HB], F32, name="ps", tag="m1")
                nc.tensor.matmul(ps[:, :], lhsT=msel[:, :],
                                 rhs=u_slots[:, (g + 1) * HB:(g + 2) * HB],
                                 start=True, stop=True)
                nc.scalar.activation(
                    out=scale_row[:, kev * HB:(kev + 1) * HB],
                    in_=ps[:, :], func=AF.Ln)
                ps_tiles = getattr(em_chunk, '_ps', {})
                ps_tiles[g] = ps
                em_chunk._ps = ps_tiles

            if g == EVENTS[0] + 2:
                # reciprocal/pb two steps after the event: their inputs are
                # long done, so the waits never stall the wait queues
                ps = em_chunk._ps.pop(EVENTS[0])
                rec = smalls.tile([2, HB], BF16, name="rec", tag="rec")
                nc.vector.reciprocal(rec[:, :], ps[:, :])
                pb = miscp.tile([NP2, HB], F32, name="pb", tag="m2")
                nc.tensor.matmul(pb[:, :], lhsT=bsel[:, :], rhs=rec[:, :],
                                 start=True, stop=False)
                nc.tensor.matmul(pb[:, :], lhsT=e2[:, :], rhs=onesrow[:, :],
                                 start=False, stop=True)
                pend[EVENTS[0] + DEF] = pb

            if g == W - 1:
                # boundary mass A from slot W-1 (issued one step after the
                # slot is written: no wait-queue stall)
                bw = miscp.tile([2, HB], F32, name="bw", tag="m1")
                nc.tensor.matmul(bw[:, :], lhsT=msel[:, :],
                                 rhs=u_slots[:, (W - 1) * HB:W * HB],
                                 start=True, stop=True)
                nc.scalar.activation(out=a_row[:, :], in_=bw[:, :], func=AF.Ln)

            if g == EVENTS[0] + 3:
                # scale-log selection (scale_row settled well before)
                scm = consts.tile([2, NEV * HB], F32, name="scm")
                nc.vector.tensor_mul(scm[:, :], scale_row[:, :], selsc)
                scred = consts.tile([2, BL], F32, name="scred")
                nc.vector.tensor_reduce(
                    out=scred[:, :],
                    in_=scm[:, :].rearrange("p (c b) -> p b c",
                                            c=NEV * CH, b=BL),
                    axis=mybir.AxisListType.X, op=ALU.add)

            if g in blk_ready:
                for q in blk_ready[g]:
                    end_block_dma(0, q)
                    end_block_dma(1, q)

        # early Ln for extraction blocks that landed during the scan
        # (0..BLKH-2 of each group); the pf-dependent blocks follow later
        endlog = consts.tile([128, NBLK * BL], F32, name="endlog")
        eb = BLKH - 1
        nc.vector.tensor_scalar_max(endbuf[:, 0:eb * BL],
                                    endbuf[:, 0:eb * BL], 1e-38)
        nc.vector.tensor_scalar_max(endbuf[:, BLKH * BL:(BLKH + eb) * BL],
                                    endbuf[:, BLKH * BL:(BLKH + eb) * BL],
                                    1e-38)
        nc.scalar.activation(out=endlog[:, 0:eb * BL],
                             in_=endbuf[:, 0:eb * BL], func=AF.Ln)
        nc.scalar.activation(out=endlog[:, BLKH * BL:(BLKH + eb) * BL],
                             in_=endbuf[:, BLKH * BL:(BLKH + eb) * BL],
                             func=AF.Ln)

        # final end-dots for states at slot LSTEPS
        pf = pp.tile([NP2, HB], F32, name="pf", tag="p")
        nc.tensor.matmul(pf[:, :], lhsT=statd[:, :],
                         rhs=u_slots[:, LSTEPS * HB:(LSTEPS + 1) * HB],
                         start=True, stop=True)
        # only the END rows of the final slot matter; copy the 32-quad
        # containing each block's END row (offset starts limited to 32 rows)
        nc.scalar.copy(u_slots[32:64, (LSTEPS + 1) * HB:(LSTEPS + 2) * HB],
                       pf[32:64, :])
        nc.vector.tensor_copy(
            u_slots[96:128, (LSTEPS + 1) * HB:(LSTEPS + 2) * HB],
            pf[96:128, :])
        end_block_dma(0, BLKH - 1, eng=nc.sync)
        end_block_dma(1, BLKH - 1, eng=nc.scalar)

        # stitch from slot LSTEPS-1 (same absolute boundary as the A-side
        # slot W-1); issued here so its semaphore waits don't occupy the
        # PE/DVE wait queues between the last scan steps
        be = miscp.tile([2, HB], F32, name="be", tag="m1")
        nc.tensor.matmul(be[:, :], lhsT=msel[:, :],
                         rhs=u_slots[:, (LSTEPS - 1) * HB:LSTEPS * HB],
                         start=True, stop=True)
        b_row = consts.tile([2, HB], F32, name="b_row")
        nc.scalar.activation(out=b_row[:, :], in_=be[:, :], func=AF.Ln)
        for kev in range(NEV):
            nc.vector.tensor_add(b_row[:, :], b_row[:, :],
                                 scale_row[:, kev * HB:(kev + 1) * HB])
        # D in [2, HB] form; the cross-group boundary element is patched
        # into kacc by a rank-1 correction matmul (lt23)
        d2 = consts.tile([2, HB], F32, name="d2")
        nc.vector.memset(d2[:, :], 0.0)
        nc.vector.tensor_sub(d2[0:2, BL:HB], b_row[0:2, 0:HB - BL],
                             a_row[0:2, BL:HB])
        dm = consts.tile([C, BL], F32, name="dm")
        nc.sync.dma_start(out=dm[:, :],
                          in_=d2[:, :].rearrange("p (c b) -> p c b",
                                                 c=CH, b=BL))
        kacc = miscp.tile([C, BL], F32, name="kacc", tag="m2")
        nc.tensor.matmul(kacc[:, :], lhsT=ltri, rhs=dm[:, :],
                         start=True, stop=False)
        nc.tensor.matmul(kacc[:, :], lhsT=lt23b, rhs=b_row[0:2, HB - BL:HB],
                         start=False, stop=False)
        nc.tensor.matmul(kacc[:, :], lhsT=lt23n, rhs=a_row[0:2, 0:BL],
                         start=False, stop=True)
        kmask = consts.tile([C, BL], F32, name="kmask")
        nc.vector.tensor_mul(kmask[:, :], kacc[:, :], selk)

        # ---------------- norm score selection ----------------
        for blk in (BLKH - 1, NBLK - 1):
            nc.vector.tensor_scalar_max(endbuf[:, blk * BL:(blk + 1) * BL],
                                        endbuf[:, blk * BL:(blk + 1) * BL],
                                        1e-38)
            nc.scalar.activation(out=endlog[:, blk * BL:(blk + 1) * BL],
                                 in_=endbuf[:, blk * BL:(blk + 1) * BL],
                                 func=AF.Ln)
        nc.vector.tensor_mul(endlog[:, :], endlog[:, :], selend)
        esum = consts.tile([128, BL], F32, name="esum")
        nc.vector.tensor_reduce(
            out=esum[:, :],
            in_=endlog[:, :].rearrange("p (blk b) -> p b blk",
                                       blk=NBLK, b=BL),
            axis=mybir.AxisListType.X, op=ALU.add)

        nacc = miscp.tile([1, BL], F32, name="nacc", tag="m1")
        nc.tensor.matmul(nacc[:, :], lhsT=onesr, rhs=esum[:, :],
                         start=True, stop=False)
        nc.tensor.matmul(nacc[:, :], lhsT=onesr[0:2, :], rhs=scred[:, :],
                         start=False, stop=False)
        nc.tensor.matmul(nacc[:, :], lhsT=onesr[0:C, :], rhs=kmask[:, :],
                         start=False, stop=False)
        nc.tensor.matmul(nacc[:, :], lhsT=onesr[0:1, :], rhs=scc,
                         start=False, stop=False)
        # negated gold pieces into column 0
        nc.tensor.matmul(nacc[:, 0:1], lhsT=ones[0:NL, :], rhs=gt1[:, :],
                         start=False, stop=False)
        nc.tensor.matmul(nacc[:, 0:1], lhsT=ones[0:L, :], rhs=gt2[:, :],
                         start=False, stop=False)
        nc.tensor.matmul(nacc[:, 0:1], lhsT=ones[0:L, :], rhs=ur[:, :],
                         start=False, stop=True)

        # loss = sum_b (norm - gold)
        dbgt = smalls.tile([1, 1], F32, name="dbgt", tag="dbgt")
        nc.vector.tensor_reduce(out=dbgt[0:1, 0:1], in_=nacc[:, :],
                                axis=mybir.AxisListType.X, op=ALU.add)
        nc.sync.dma_start(out=loss_d.ap()[:, :], in_=dbgt[:, :])

    nc.compile()
    return nc


def prep_inputs(inputs, W_in, b, transition, lens, labels):
    """Host-side sharding + index preprocessing. Returns per-core input maps."""
    x = np.ascontiguousarray(np.asarray(inputs, dtype=np.float32))
    Wm = np.asarray(W_in, dtype=np.float32)
    b = np.asarray(b, dtype=np.float32)
    T = np.asarray(transition, dtype=np.float32)
    lens = np.asarray(lens).astype(np.int64)
    labels = np.asarray(labels).astype(np.int64)

    # W duplicated in both partition blocks, scaled for fp8
    WT = np.zeros((KD, 128, NP2), dtype=np.float32)
    Wk = np.ascontiguousarray((Wm * SW).T).reshape(KD, 128, L)
    WT[:, :, 0:L] = Wk
    WT[:, :, NLB:NLB + L] = Wk
    WT8 = WT.astype(ml_dtypes.float8_e4m3)
    TT = np.ascontiguousarray(T.T)

    xt_all = np.ascontiguousarray(np.transpose(x, (2, 1, 0)))  # (D, S, B)

    ELI = np.zeros((14, LSTEPS * HB), dtype=np.float32)
    ELI[1, :] = 1.0
    UINIT = np.zeros((NP2, HB), dtype=np.float32)
    UINIT[START, 0:BL] = 1.0                       # chunk 0: true init
    UINIT[:L, BL:] = 1.0 / L                       # group A probes
    UINIT[NLB:NLB + L, :] = 1.0 / L                # group B probes
    UINIT = UINIT.astype(ml_dtypes.bfloat16)
    LTRI = np.zeros((C, C), dtype=np.float32)
    for cc in range(C):
        LTRI[:cc + 1, cc] = 1.0

    in_maps = []
    for core in range(NCORES):
        bs = slice(core * BL, (core + 1) * BL)
        lens_c = lens[bs]
        labels_c = labels[bs]

        xT = np.ascontiguousarray(xt_all[:, :, bs]).reshape(KD, 128, S * BL)
        xT8 = xT.astype(ml_dtypes.float8_e4m3)

        mask = np.arange(S)[:, None] < lens_c[None, :]
        lab_t = labels_c.T
        OH = (lab_t[None, :, :] == np.arange(L)[:, None, None]) & mask[None]
        OH = np.ascontiguousarray(
            OH.reshape(L, S * BL)).astype(ml_dtypes.bfloat16)

        ext = np.full((BL, S + 2), END, dtype=np.int64)
        ext[:, 0] = START
        ext[:, 1:S + 1] = labels_c
        valid = np.arange(S + 2)[None, :] < (lens_c + 1)[:, None]
        ext = np.where(valid, ext, END)
        CNT = np.zeros((NL, NL), dtype=np.float32)
        pmask = np.arange(S + 1)[None, :] < (lens_c + 1)[:, None]
        np.add.at(CNT, (ext[:, 1:][pmask], ext[:, :-1][pmask]), 1.0)

        CNTL = np.zeros((L,), dtype=np.float32)
        msk = np.arange(S)[None, :] < lens_c[:, None]
        np.add.at(CNTL, labels_c[msk], 1.0)

        SELEND = np.zeros((128, NBLK * BL), dtype=np.float32)
        SELSC = np.zeros((2, NEV * HB), dtype=np.float32)
        SELK = np.zeros((C, BL), dtype=np.float32)
        for bb in range(BL):
            l = int(lens_c[bb])
            cch = 0 if l <= W + G else (l - W - 1) // G
            gg = l - cch * G
            hb, cm = divmod(cch, CH)
            pi = (gg + 1) * CH + cm - CH          # flat pair idx within group
            blk, row = divmod(pi, 128)
            SELEND[row, (hb * BLKH + blk) * BL + bb] = 1.0
            for kev in range(NEV):
                if EVENTS[kev] + DEF <= gg - 1:
                    SELSC[hb, (kev * CH + cm) * BL + bb] = 1.0
            SELK[cch, bb] = 1.0
        SCC = C0 * lens_c.astype(np.float32)

        CPK = np.zeros((128, CPW), dtype=np.float32)
        CPK[0:NL, O_TT:O_TT + NL] = TT
        CPK[0:NL, O_T:O_T + NL] = T
        CPK[0:NL, O_CNT:O_CNT + NL] = -CNT
        CPK[0:L, O_CNTL] = -CNTL
        CPK[0:L, O_BIAS] = b
        CPK[0:L, O_BIASC] = b - C0
        CPK[NLB:NLB + L, O_BIASC] = b - C0
        CPK[:, O_SELEND:O_SELEND + NBLK * BL] = SELEND
        CPK[0:2, O_SELSC:O_SELSC + NEV * HB] = SELSC
        CPK[0:C, O_SELK:O_SELK + BL] = SELK
        CPK[0:C, O_LTRI:O_LTRI + C] = LTRI
        CPK[0:1, O_SCC:O_SCC + BL] = SCC
        CPK[0, O_LT23B + CH:O_LT23B + C] = 1.0
        CPK[1, O_LT23N + CH:O_LT23N + C] = -1.0
        CPK[:, O_ONES] = 1.0

        BSEL = np.zeros((2, NP2), dtype=np.float32)
        BSEL[0, 0:L] = 1.0
        BSEL[1, NLB:NLB + L] = 1.0
        in_maps.append({
            "xT": xT8, "WT": WT8, "OH": OH,
            "UINIT": UINIT, "ELI": ELI, "CPK": CPK,
            "BSEL": BSEL.astype(ml_dtypes.bfloat16),
        })
    return in_maps


_NC_CACHE = []


def kernel(inputs, W, b, transition, lens, labels, _trace=False, _tmpdir=None):
    in_maps = prep_inputs(inputs, W, b, transition, lens, labels)
    if not _NC_CACHE:
        _NC_CACHE.append(build_program())
    nc = _NC_CACHE[0]
    res = run_bass_kernel_spmd(nc, in_maps, list(range(NCORES)),
                               trace=_trace, tmpdir=_tmpdir)
    total = np.float64(0.0)
    for r in res.results:
        total += np.float64(r["loss"][0, 0])
    out = np.float32(total)
    if _trace:
        return out, res
    return out
